# revision 3
# baseline (speedup 1.0000x reference)
"""Bass/Trainium2 kernel for DynamicMultiheadAttention (sparse_attention).

v2: attention@V runs in TRANSPOSED orientation — oT[n, c] = p2.T @ v with
p2 (scores) as the PE stationary and v ([128, 65] incl. ones column) as the
moving tensor.  PE matmul cost is output-free-size cycles, so the o-path
drops from 120x[65,512] (213ns) to 480x[128,65] (27ns): ~-12.6us PE.
The softmax rowsum lands as PSUM column 64 per (n-partition, head), so
normalization is a per-partition tensor_scalar DIVIDE (no reciprocal
broadcasts, no ones2 matmuls, no ACT copies).  The normalized oT is
PE-transposed back ([n,dh] -> [dh,n], 16x 53ns) to feed the output
projection, which is unchanged.

Sharding: 8 cores = (batch b in {0,1}) x (query-slice of 512 rows).
Each core computes all 8 heads for its (b, n-slice); scores sT[m, n]
with keys m on partitions.

The relative-mask bias is applied multiplicatively after the exp:
  exp(s + rel) = exp(s) * E,  E[h,m,n] = exp(-sum_r c[h,r]*attn_mask)
E is precomputed on the host as bf16 planes; the all-16-bit
tensor_tensor multiply runs in the DVE 2x_1p fast mode (~593ns per
two-head tile).

Fully-padded key tiles are skipped at program-build time; partially
padded tiles are handled by zeroing the affected rows of V and of the
appended ones-column.  The k bias is softmax-invariant and dropped; the
v bias folds into the output bias: bo' = bv @ Wo + bo.

Every TPB instruction encoding in this walrus build tolerates only ONE
semaphore wait; a post-pass (_split_matmul_waits) moves extra waits onto
standalone single-wait EventSemaphore instructions.
"""

import numpy as np
import ml_dtypes
import os

def _B(name, default):
    return int(os.environ.get("KB_" + name, default))

N, B, D = 2048, 2, 512
H, R = 8, 3
C = D // H          # 64
NS = N // 4         # 512 query rows per core
NCORES = 8
MT = N // 128       # 16 key tiles

_cache = {}


def _build_program(active, reps=1):
    import concourse.bass as bass
    import concourse.mybir as mybir
    import concourse.tile as tile
    from contextlib import ExitStack

    f32 = mybir.dt.float32
    f32r = mybir.dt.float32r
    bf16 = mybir.dt.bfloat16
    u8 = mybir.dt.uint8
    AFT = mybir.ActivationFunctionType
    ALU = mybir.AluOpType

    MTA = len(active)

    nc = bass.Bass()

    xtq = nc.declare_dram_parameter("xtq", [D, NS], bf16, isOutput=False)
    xtk = nc.declare_dram_parameter("xtk", [D, N], bf16, isOutput=False)
    xtv = nc.declare_dram_parameter("xtv", [D, N], bf16, isOutput=False)
    # E planes, partition-major: [hp, p(=m%128), mi, j(head in pair), n]
    epl = nc.declare_dram_parameter("epl", [H // 2, 128, MTA, 2, NS], bf16,
                                    isOutput=False)
    wq = nc.declare_dram_parameter("wq", [D, D], bf16, isOutput=False)
    wk = nc.declare_dram_parameter("wk", [D, D], bf16, isOutput=False)
    wv = nc.declare_dram_parameter("wv", [D, D], bf16, isOutput=False)
    wo = nc.declare_dram_parameter("wo", [D, D], bf16, isOutput=False)
    bq2 = nc.declare_dram_parameter("bq2", [128, 4], f32, isOutput=False)
    bo2 = nc.declare_dram_parameter("bo2", [128, 4], f32, isOutput=False)
    # per-active-tile pad multiplier planes (zero padded key rows of V)
    pad = nc.declare_dram_parameter("pad", [128, MTA], f32, isOutput=False)
    pad8 = nc.declare_dram_parameter("pad8", [128, MTA, H], f32, isOutput=False)
    ident = nc.declare_dram_parameter("ident", [128, 128], bf16, isOutput=False)
    outT = nc.declare_dram_parameter("outT", [D, NS], bf16, isOutput=True)

    with tile.TileContext(nc) as tc, ExitStack() as ctx:
        mm = nc.tensor.matmul
        _run_once(nc, tc, ctx, mm, tile, mybir, f32, f32r, bf16, u8,
                  AFT, ALU, active, xtq, xtk, xtv, epl, wq, wk, wv, wo,
                  bq2, bo2, pad, pad8, ident, outT)

    _split_matmul_waits(nc, mybir)
    return nc


def _run_once(nc, tc, ctx, mm, tile, mybir, f32, f32r, bf16, u8, AFT, ALU,
              active, xtq, xtk, xtv, epl, wq, wk, wv, wo, bq2, bo2,
              pad, pad8, ident, outT):
    from contextlib import ExitStack
    MTA = len(active)
    with ExitStack() as ctx:
        const_pool = ctx.enter_context(tc.tile_pool(name="const", bufs=1))
        persist = ctx.enter_context(tc.tile_pool(name="persist", bufs=1))

        # constants ride the Pool queue: SP's 650ns-per-DMA dispatch rate is
        # the lead-in bottleneck, so it is reserved for the q/k-path inputs
        loads = []
        bq_sb = const_pool.tile([128, 4], f32)
        loads.append(nc.gpsimd.dma_start(bq_sb[:], bq2[:]))
        bo_sb = const_pool.tile([128, 4], f32)
        loads.append(nc.gpsimd.dma_start(bo_sb[:], bo2[:]))
        pad_sb = const_pool.tile([128, MTA], f32)
        loads.append(nc.gpsimd.dma_start(pad_sb[:], pad[:]))
        pad8_sb = const_pool.tile([128, MTA, H], f32)
        loads.append(nc.gpsimd.dma_start(pad8_sb[:], pad8[:]))
        ident_sb = const_pool.tile([128, 128], bf16)
        loads.append(nc.gpsimd.dma_start(ident_sb[:], ident[:]))
        wo_sb = persist.tile([128, 4, D], bf16)

        kT_sb = persist.tile([128, 4, N], bf16)
        qT_sb = persist.tile([128, 4, NS], bf16)
        v_sb = persist.tile([128, MTA, H, C + 1], bf16)
        OT_sb = persist.tile([128, 4, NS], bf16)
        # normalized oT per pass: [n, nc4, dh-block, head-in-block, c]
        OTn_sb = persist.tile([128, 4, 2, 2, C], bf16)
        outT_sb = persist.tile([128, 4, NS], bf16)

        # ---- Phase A (part 1): DMAs + projections needed by pass 0 ----
        xw_pool = ctx.enter_context(tc.tile_pool(name="xw", bufs=1))
        wq_sb = xw_pool.tile([128, 4, D], bf16, tag="w")
        wk_sb = xw_pool.tile([128, 4, D], bf16, tag="w2")
        wv_sb = xw_pool.tile([128, 4, D], bf16, tag="w3")
        xtq_sb = xw_pool.tile([128, 4, NS], bf16, tag="xq")
        xtk_sb = xw_pool.tile([128, 4, N], bf16, tag="xk")
        xtv_sb = xw_pool.tile([128, 4, N], bf16, tag="xv")
        # q/k path on SP, v path on the Pool queue: parallel dispatch halves
        # the dispatch-bound lead-in.
        nc.sync.dma_start(wq_sb[:, :, 0:256],
                          wq[:, 0:256].rearrange("(c p) d -> p c d", p=128))
        nc.sync.dma_start(xtq_sb[:, 0:2, :],
                          xtq[0:256].rearrange("(c p) n -> p c n", p=128))
        nc.sync.dma_start(xtq_sb[:, 2:4, :],
                          xtq[256:512].rearrange("(c p) n -> p c n", p=128))
        nc.sync.dma_start(wk_sb[:, :, 0:256],
                          wk[:, 0:256].rearrange("(c p) d -> p c d", p=128))
        nc.gpsimd.dma_start(wv_sb[:],
                            wv[:].rearrange("(c p) d -> p c d", p=128))
        for mb in range(4):
            sl = slice(mb * 512, (mb + 1) * 512)
            nc.sync.dma_start(
                xtk_sb[:, :, sl],
                xtk[:, sl].rearrange("(kc p) m -> p kc m", p=128))
            nc.gpsimd.dma_start(
                xtv_sb[:, :, sl],
                xtv[:, sl].rearrange("(kc p) m -> p kc m", p=128))

        nc.sync.dma_start(wq_sb[:, :, 256:512],
                          wq[:, 256:512].rearrange("(c p) d -> p c d", p=128))
        nc.sync.dma_start(wk_sb[:, :, 256:512],
                          wk[:, 256:512].rearrange("(c p) d -> p c d", p=128))

        # E-plane tiles: quarter planes [128, 4, 2, NS] bf16 per fetch
        ep_pool = ctx.enter_context(tc.tile_pool(name="ep", bufs=_B("EP", 6)))
        pT_pool = ctx.enter_context(tc.tile_pool(name="pT", bufs=_B("PT", 5)))
        p2_pool = ctx.enter_context(tc.tile_pool(name="p2", bufs=_B("P2", 5)))
        EH = 4
        ep_tiles = {}

        def fetch_e(hp, q, eng=None):
            h0 = q * EH
            hn = min(EH, MTA - h0)
            t = ep_pool.tile([128, EH, 2, NS], bf16, tag="ep")
            (eng or nc.gpsimd).dma_start(t[:, 0:hn, :, :],
                                         epl[hp, :, h0:h0 + hn, :, :])
            ep_tiles[(hp, q)] = t

        fetch_e(0, 0)
        fetch_e(1, 0)
        fetch_e(0, 1)
        fetch_e(1, 1)
        fetch_e(0, 2)
        fetch_e(1, 2)

        vones = [nc.vector.tensor_copy(
            v_sb[:, :, :, C : C + 1],
            pad8_sb[:, :, :].rearrange("p m (h o) -> p m h o", o=1))]

        with tc.tile_pool(name="psA", bufs=_B("PSA", 8), space="PSUM") as psA:
            # qT[dh, n] = (Wq/8).T @ xT_q  (+ bq/8 per-partition), heads 0-3
            for j in range(2):
                ps = psA.tile([128, NS], f32, tag="psA")
                for kc in range(4):
                    mm(ps[:], wq_sb[:, kc, j * 128:(j + 1) * 128],
                       xtq_sb[:, kc, :], start=(kc == 0), stop=(kc == 3))
                nc.scalar.activation(qT_sb[:, j, :], ps[:], AFT.Identity,
                                     bias=bq_sb[:, j:j + 1])

            # kT[dh, m] = Wk.T @ xT_k, heads 0-3 (k bias drops in softmax)
            for mb in range(4):
                for j in range(2):
                    ps = psA.tile([128, NS], f32, tag="psA")
                    for kc in range(4):
                        mm(ps[:], wk_sb[:, kc, j * 128:(j + 1) * 128],
                           xtk_sb[:, kc, mb * 512:(mb + 1) * 512],
                           start=(kc == 0), stop=(kc == 3))
                    if (mb + j) % 2 == 0:
                        nc.scalar.copy(kT_sb[:, j, mb * 512:(mb + 1) * 512],
                                       ps[:])
                    else:
                        nc.vector.tensor_copy(
                            kT_sb[:, j, mb * 512:(mb + 1) * 512], ps[:])

            # v[m, c] = xT_v.T @ Wv, padded key rows zeroed (scale by pad01)
            for mi, mt in enumerate(active):
                ps = psA.tile([128, D], f32, tag="psA")
                for kc in range(4):
                    mm(ps[:], xtv_sb[:, kc, mt * 128:(mt + 1) * 128],
                       wv_sb[:, kc, :], start=(kc == 0), stop=(kc == 3))
                nc.vector.tensor_scalar(
                    v_sb[:, mi, :, 0:C],
                    ps[:].rearrange("p (h c) -> p h c", h=H),
                    pad_sb[:, mi:mi + 1], None, ALU.mult)

        # PSUM pools for phase B (psA released its banks above)
        # psO-tag ring: 4 slots of 1 bank each.  Holds in turn: the o
        # accumulators ([128, 4, 65] f32: 4 heads + rowsum col per n-chunk),
        # the deferred j23 k projections ([128, 512] f32), and the output
        # projection tiles.  psS tiles are [128, 2, NS] (2 banks).
        small_pool = ctx.enter_context(tc.tile_pool(name="small", bufs=4))
        psO = ctx.enter_context(tc.tile_pool(name="psO", bufs=4, space="PSUM"))
        psS = ctx.enter_context(tc.tile_pool(name="psS", bufs=_B("PSS", 2), space="PSUM"))

        # ---- Phase B: attention, two passes of 4 heads (2 head pairs) ----
        def attn_pass(p, hooks={}):
            o_ps = [psO.tile([128, 4, C + 1], f32, tag="psO",
                             name=f"o_ps{p}_{i}") for i in range(4)]

            def emit_o(p2, mi, hpl):
                for j in range(2):
                    h = 4 * p + 2 * hpl + j
                    hl = 2 * hpl + j
                    for nc4 in range(4):
                        mm(o_ps[nc4][:, hl, :],
                           p2[:, j, nc4 * 128:(nc4 + 1) * 128],
                           v_sb[:, mi, h, :],
                           start=(mi == 0 and hl == 0),
                           stop=(mi == MTA - 1 and hl == 3),
                           skip_group_check=True)

            pending = []
            for mi in range(MTA):
                for fn in hooks.get(mi, ()):
                    fn()
                for hpl in range(2):
                    hp = 2 * p + hpl
                    if p == 0 and hpl == 0 and mi in (6, 8, 10, 12):
                        nq = {6: [(0, 3), (1, 3)], 8: [(2, 0), (3, 0)],
                              10: [(2, 1), (3, 1)], 12: [(2, 2), (3, 2)]}[mi]
                        for a, b in nq:
                            fetch_e(a, b)
                    if p == 1 and hpl == 0 and mi == 4:
                        fetch_e(2, 3)
                        fetch_e(3, 3)
                    s_ps = psS.tile([128, 2, NS], f32, tag="psS")
                    for j in range(2):
                        h = 4 * p + 2 * hpl + j
                        hj, ho = h // 2, (h % 2) * 64
                        mm(s_ps[:, j, :],
                           kT_sb[ho:ho + 64, hj, active[mi] * 128:active[mi] * 128 + 128],
                           qT_sb[ho:ho + 64, hj, :], start=True, stop=True)
                    pT = pT_pool.tile([128, 2, NS], bf16, tag="pT")
                    nc.scalar.activation(pT[:], s_ps[:], AFT.Exp)
                    p2 = p2_pool.tile([128, 2, NS], bf16, tag="p2")
                    # all-bf16 tensor_tensor: DVE 2x_1p fast mode (~593ns)
                    nc.vector.tensor_tensor(
                        p2[:], pT[:],
                        ep_tiles[(hp, mi // EH)][:, mi % EH, :, :], ALU.mult)
                    # transposed o accumulation: oT[n, c] += p2[m, n-chunk].T
                    # @ v[m, c|1].  One accumulation group per PSUM bank
                    # (bank-wide zero region): start on the bank's first
                    # write, stop on its last.
                    for j in range(2):
                        h = 4 * p + 2 * hpl + j
                        hl = 2 * hpl + j
                        for nc4 in range(4):
                            mm(o_ps[nc4][:, hl, :],
                               p2[:, j, nc4 * 128:(nc4 + 1) * 128],
                               v_sb[:, mi, h, :],
                               start=(mi == 0 and hl == 0),
                               stop=(mi == MTA - 1 and hl == 3),
                               skip_group_check=True)
            return o_ps

        def normalize(p, o_ps, halves=(0, 1), nc4s=(0, 1, 2, 3)):
            # oT[n, c] * (1/rowsum[n]): per-partition reciprocal (DVE) +
            # scalar multiply, split DVE/ACT (ACT is idle at pass ends;
            # tensor_scalar divide fails the walrus ISA check).
            for half in halves:
                for k, hl in enumerate((2 * half, 2 * half + 1)):
                    for nc4 in nc4s:
                        rec = small_pool.tile([128, 1], f32, tag="rec",
                                              name=f"rc{p}_{hl}_{nc4}")
                        nc.vector.reciprocal(rec[:], o_ps[nc4][:, hl, C:C + 1])
                        if (nc4 + k) % 2 == 0:
                            nc.vector.tensor_scalar(
                                OTn_sb[:, nc4, half, hl % 2, :],
                                o_ps[nc4][:, hl, 0:C],
                                rec[:], None, ALU.mult)
                        else:
                            nc.scalar.activation(
                                OTn_sb[:, nc4, half, hl % 2, :],
                                o_ps[nc4][:, hl, 0:C],
                                AFT.Identity, scale=rec[:])

        def transposes(p, b, eng):
            # [n, dh-block] -> [dh-block, n] via PE transpose, then one
            # 512-wide evacuation copy into OT_sb[:, 2p+b, :].
            tp = psS.tile([128, 4, 128], bf16, tag="psS", name=f"tp{p}_{b}")
            for nc4 in range(4):
                nc.tensor.transpose(tp[:, nc4, :], OTn_sb[:, nc4, b, :, :],
                                    ident_sb[:])
            if eng is nc.scalar:
                eng.copy(OT_sb[:, 2 * p + b, :],
                         tp[:].rearrange("p a n -> p (a n)"))
            else:
                eng.tensor_copy(OT_sb[:, 2 * p + b, :],
                                tp[:].rearrange("p a n -> p (a n)"))

        # deferred projections for heads 4-7 (j-blocks 2,3)
        def proj_j23_k_psS(mb):
            ps = psS.tile([128, 2, NS], f32, tag="psS", name=f"kp{mb}")
            for j in (2, 3):
                for kc in range(4):
                    mm(ps[:, j - 2, :], wk_sb[:, kc, j * 128:(j + 1) * 128],
                       xtk_sb[:, kc, mb * 512:(mb + 1) * 512],
                       start=(kc == 0), stop=(kc == 3))
            nc.scalar.copy(kT_sb[:, 2, mb * 512:(mb + 1) * 512], ps[:, 0, :])
            nc.vector.tensor_copy(
                kT_sb[:, 3, mb * 512:(mb + 1) * 512], ps[:, 1, :])

        def proj_j23_q_psS():
            ps = psS.tile([128, 2, NS], f32, tag="psS", name="qp23")
            for j in (2, 3):
                for kc in range(4):
                    mm(ps[:, j - 2, :], wq_sb[:, kc, j * 128:(j + 1) * 128],
                       xtq_sb[:, kc, :], start=(kc == 0), stop=(kc == 3))
            for j in (2, 3):
                nc.scalar.activation(qT_sb[:, j, :], ps[:, j - 2, :],
                                     AFT.Identity, bias=bq_sb[:, j:j + 1])

        def proj_j23_k_psO(mb):
            for j in (2, 3):
                ps = psO.tile([128, NS], f32, tag="psO", name=f"kp{mb}_{j}")
                for kc in range(4):
                    mm(ps[:], wk_sb[:, kc, j * 128:(j + 1) * 128],
                       xtk_sb[:, kc, mb * 512:(mb + 1) * 512],
                       start=(kc == 0), stop=(kc == 3))
                if j == 2:
                    nc.scalar.copy(
                        kT_sb[:, j, mb * 512:(mb + 1) * 512], ps[:])
                else:
                    nc.vector.tensor_copy(
                        kT_sb[:, j, mb * 512:(mb + 1) * 512], ps[:])

        o_ps0 = attn_pass(0)
        proj_j23_k_psS(0)
        proj_j23_q_psS()
        # both normalize halves must finish reading the o banks before any
        # psO-ring reuse: each bank holds all four heads of the pass.
        normalize(0, o_ps0, (0,))
        normalize(0, o_ps0, (1,))
        transposes(0, 0, nc.vector)
        transposes(0, 1, nc.scalar)
        proj_j23_k_psO(1)
        proj_j23_k_psO(2)
        proj_j23_k_psO(3)
        for c in range(4):
            nc.sync.dma_start(wo_sb[:, c, :], wo[c * 128:(c + 1) * 128, :])

        o_ps1 = attn_pass(1)

        # ---- Phase C: output projection, pipelined with pass-1 tail ----
        def outproj_partial(jt):
            ps = psO.tile([128, NS], f32, tag="psO", name=f"oc{jt}")
            for g in (0, 1, 2):
                mm(ps[:], wo_sb[:, g, jt * 128:(jt + 1) * 128],
                   OT_sb[:, g, :], start=(g == 0), stop=False)
            return ps

        def outproj_finish(jt, ps=None):
            if ps is None:
                ps = psO.tile([128, NS], f32, tag="psO", name=f"oc{jt}")
                for g in (0, 1, 2):
                    mm(ps[:], wo_sb[:, g, jt * 128:(jt + 1) * 128],
                       OT_sb[:, g, :], start=(g == 0), stop=False)
            mm(ps[:], wo_sb[:, 3, jt * 128:(jt + 1) * 128],
               OT_sb[:, 3, :], start=False, stop=True)
            if jt % 2 == 0:
                nc.scalar.activation(outT_sb[:, jt, :], ps[:], AFT.Identity,
                                     bias=bo_sb[:, jt:jt + 1])
            else:
                nc.vector.tensor_scalar(outT_sb[:, jt, :], ps[:],
                                        bo_sb[:, jt:jt + 1], None, ALU.add)
            nc.sync.dma_start(outT[jt * 128:(jt + 1) * 128, :],
                              outT_sb[:, jt, :])

        normalize(1, o_ps1, (0,))
        normalize(1, o_ps1, (1,))
        transposes(1, 0, nc.vector)
        pc0 = outproj_partial(0)
        pc1 = outproj_partial(1)
        transposes(1, 1, nc.scalar)
        outproj_finish(0, pc0)
        outproj_finish(1, pc1)
        outproj_finish(2)
        outproj_finish(3)


# every TPB instruction encoding in this walrus build tolerates only a
# single semaphore wait -- split extras regardless of opcode
_NO_SPLIT_TYPES = {"InstEventSemaphore"}


def _split_matmul_waits(nc, mybir):
    """Several engine instruction encodings tolerate only one semaphore
    wait; move extra waits onto standalone single-wait EventSemaphore
    instructions inserted right before them on the same engine queue."""
    import bass_rust

    n = 0
    for bb in nc.m.functions[0].blocks:
        insts = list(bb.instructions)
        out = []
        changed = False
        for i in insts:
            si = i.sync_info
            if (type(i).__name__ not in _NO_SPLIT_TYPES and si is not None
                    and len(si.on_wait) > 1):
                w = list(si.on_wait)
                for wx in w[:-1]:
                    ev = mybir.InstEventSemaphore(name=f"mmw_{n}_{i.name}",
                                                  ins=[], outs=[])
                    ev.engine = i.engine
                    ev.sync_info = bass_rust.SyncInfo(on_wait=[wx],
                                                      on_update=[])
                    out.append(ev)
                    n += 1
                si.on_wait = [w[-1]]
                changed = True
            out.append(i)
        if changed:
            bb.instructions = out


def _host_prep(inputs):
    x_q = np.asarray(inputs["x_q"], np.float32)
    x_k = np.asarray(inputs["x_k"], np.float32)
    x_v = np.asarray(inputs["x_v"], np.float32)
    attn_mask = np.asarray(inputs["attn_mask"]).astype(bool)
    kpm = np.asarray(inputs["key_padding_mask"]).astype(bool)
    Wq = np.asarray(inputs["Wq"], np.float32)
    Wk = np.asarray(inputs["Wk"], np.float32)
    Wv = np.asarray(inputs["Wv"], np.float32)
    Wo = np.asarray(inputs["Wo"], np.float32)
    bq = np.asarray(inputs["bq"], np.float32)
    bv = np.asarray(inputs["bv"], np.float32)
    bo = np.asarray(inputs["bo"], np.float32)
    mw = np.asarray(inputs["mask_weight"], np.float64)

    # c[h,r] = softmax(mask_weight[h,:R]) * mask_weight[h,R]
    e = np.exp(mw[:, :R] - mw[:, :R].max(axis=1, keepdims=True))
    w = e / e.sum(axis=1, keepdims=True)
    c = (w * mw[:, R:R + 1]).astype(np.float32)          # [H, R]

    # active key tiles (at least one unpadded key) -- shared across batch
    tile_padded = kpm.reshape(B, MT, 128).all(axis=2)    # [B, MT]
    active = [mt for mt in range(MT) if not tile_padded[:, mt].all()]
    MTA = len(active)

    scale = np.float32(1.0 / np.sqrt(C))
    wq_s = (Wq * scale).astype(np.float32)
    bq_s = (bq * scale).astype(np.float32)
    bo_p = (bv @ Wo + bo).astype(np.float32)

    bq2 = np.ascontiguousarray(bq_s.reshape(4, 128).T)
    bo2 = np.ascontiguousarray(bo_p.reshape(4, 128).T)

    bf = ml_dtypes.bfloat16
    common = dict(wq=wq_s.astype(bf), wk=Wk.astype(bf), wv=Wv.astype(bf),
                  wo=Wo.astype(bf), bq2=bq2, bo2=bo2,
                  ident=np.eye(128, dtype=bf))

    emul = np.exp(-c)                                    # [H, R] in (0,1]
    in_maps = []
    for core in range(NCORES):
        b, ns = core // 4, core % 4
        n0 = ns * NS
        pad01 = (~kpm[b]).astype(np.float32)             # [N]
        pad2 = np.ascontiguousarray(
            pad01.reshape(MT, 128).T[:, active])         # [128, MTA]
        pad8 = np.ascontiguousarray(np.repeat(pad2[:, :, None], H, axis=2))
        inv = attn_mask[b, :, n0:n0 + NS, :]             # [R, NS, N]
        ep = np.empty((H // 2, 128, MTA, 2, NS), bf)
        for mi, mt in enumerate(active):
            invt = inv[:, :, mt * 128:(mt + 1) * 128]    # [R, NS, 128]
            bias = np.einsum('hr,rnm->hmn', c, invt.astype(np.float32))
            ep[:, :, mi] = np.exp(-bias).astype(bf).reshape(
                H // 2, 2, 128, NS).transpose(0, 2, 1, 3)
        ep = np.ascontiguousarray(ep)
        m = dict(common)
        m["xtq"] = np.ascontiguousarray(x_q[n0:n0 + NS, b, :].T).astype(bf)
        m["xtk"] = np.ascontiguousarray(x_k[:, b, :].T).astype(bf)
        m["xtv"] = np.ascontiguousarray(x_v[:, b, :].T).astype(bf)
        m["epl"] = ep
        m["pad"] = pad2
        m["pad8"] = pad8
        in_maps.append(m)
    return in_maps, active


def kernel(**inputs) -> np.ndarray:
    from concourse.bass_utils import run_bass_kernel_spmd

    in_maps, active = _host_prep(inputs)
    key = tuple(active)
    if key not in _cache:
        _cache[key] = _build_program(active)
        _cache["nc"] = _cache[key]
    nc = _cache[key]

    res = run_bass_kernel_spmd(nc, in_maps, list(range(NCORES)))

    out = np.empty((N, B, D), np.float32)
    for core in range(NCORES):
        b, ns = core // 4, core % 4
        n0 = ns * NS
        out[n0:n0 + NS, b, :] = res.results[core]["outT"].T.astype(np.float32)
    return out


# revision 5
# speedup vs baseline: 1.0203x; 1.0203x over previous
"""Bass/Trainium2 kernel for DynamicMultiheadAttention (sparse_attention).

v2: attention@V runs in TRANSPOSED orientation — oT[n, c] = p2.T @ v with
p2 (scores) as the PE stationary and v ([128, 65] incl. ones column) as the
moving tensor.  PE matmul cost is output-free-size cycles, so the o-path
drops from 120x[65,512] (213ns) to 480x[128,65] (27ns): ~-12.6us PE.
The softmax rowsum lands as PSUM column 64 per (n-partition, head), so
normalization is a per-partition tensor_scalar DIVIDE (no reciprocal
broadcasts, no ones2 matmuls, no ACT copies).  The normalized oT is
PE-transposed back ([n,dh] -> [dh,n], 16x 53ns) to feed the output
projection, which is unchanged.

Sharding: 8 cores = (batch b in {0,1}) x (query-slice of 512 rows).
Each core computes all 8 heads for its (b, n-slice); scores sT[m, n]
with keys m on partitions.

The relative-mask bias is applied multiplicatively after the exp:
  exp(s + rel) = exp(s) * E,  E[h,m,n] = exp(-sum_r c[h,r]*attn_mask)
E is precomputed on the host as bf16 planes; the all-16-bit
tensor_tensor multiply runs in the DVE 2x_1p fast mode (~593ns per
two-head tile).

Fully-padded key tiles are skipped at program-build time; partially
padded tiles are handled by zeroing the affected rows of V and of the
appended ones-column.  The k bias is softmax-invariant and dropped; the
v bias folds into the output bias: bo' = bv @ Wo + bo.

Every TPB instruction encoding in this walrus build tolerates only ONE
semaphore wait; a post-pass (_split_matmul_waits) moves extra waits onto
standalone single-wait EventSemaphore instructions.
"""

import numpy as np
import ml_dtypes
import os

def _B(name, default):
    return int(os.environ.get("KB_" + name, default))

N, B, D = 2048, 2, 512
H, R = 8, 3
C = D // H          # 64
NS = N // 4         # 512 query rows per core
NCORES = 8
MT = N // 128       # 16 key tiles

_cache = {}


def _build_program(active, reps=1):
    import concourse.bass as bass
    import concourse.mybir as mybir
    import concourse.tile as tile
    from contextlib import ExitStack

    f32 = mybir.dt.float32
    f32r = mybir.dt.float32r
    bf16 = mybir.dt.bfloat16
    u8 = mybir.dt.uint8
    AFT = mybir.ActivationFunctionType
    ALU = mybir.AluOpType

    MTA = len(active)

    nc = bass.Bass()

    xtq = nc.declare_dram_parameter("xtq", [D, NS], bf16, isOutput=False)
    xtk = nc.declare_dram_parameter("xtk", [D, N], bf16, isOutput=False)
    xtv = nc.declare_dram_parameter("xtv", [D, N], bf16, isOutput=False)
    # E planes, partition-major: [hp, p(=m%128), mi, j(head in pair), n]
    epl = nc.declare_dram_parameter("epl", [H // 2, 128, MTA, 2, NS], bf16,
                                    isOutput=False)
    wq = nc.declare_dram_parameter("wq", [D, D], bf16, isOutput=False)
    wk = nc.declare_dram_parameter("wk", [D, D], bf16, isOutput=False)
    wv = nc.declare_dram_parameter("wv", [D, D], bf16, isOutput=False)
    wo = nc.declare_dram_parameter("wo", [D, D], bf16, isOutput=False)
    bq2 = nc.declare_dram_parameter("bq2", [128, 4], f32, isOutput=False)
    bo2 = nc.declare_dram_parameter("bo2", [128, 4], f32, isOutput=False)
    # per-active-tile pad multiplier planes (zero padded key rows of V)
    pad = nc.declare_dram_parameter("pad", [128, MTA], f32, isOutput=False)
    pad8 = nc.declare_dram_parameter("pad8", [128, MTA, H], f32, isOutput=False)
    ident = nc.declare_dram_parameter("ident", [128, 128], bf16, isOutput=False)
    outT = nc.declare_dram_parameter("outT", [D, NS], bf16, isOutput=True)

    with tile.TileContext(nc) as tc, ExitStack() as ctx:
        mm = nc.tensor.matmul
        _run_once(nc, tc, ctx, mm, tile, mybir, f32, f32r, bf16, u8,
                  AFT, ALU, active, xtq, xtk, xtv, epl, wq, wk, wv, wo,
                  bq2, bo2, pad, pad8, ident, outT)

    _split_matmul_waits(nc, mybir)
    return nc


def _run_once(nc, tc, ctx, mm, tile, mybir, f32, f32r, bf16, u8, AFT, ALU,
              active, xtq, xtk, xtv, epl, wq, wk, wv, wo, bq2, bo2,
              pad, pad8, ident, outT):
    from contextlib import ExitStack
    MTA = len(active)
    with ExitStack() as ctx:
        const_pool = ctx.enter_context(tc.tile_pool(name="const", bufs=1))
        persist = ctx.enter_context(tc.tile_pool(name="persist", bufs=1))

        # constants ride the Pool queue: SP's 650ns-per-DMA dispatch rate is
        # the lead-in bottleneck, so it is reserved for the q/k-path inputs
        loads = []
        bq_sb = const_pool.tile([128, 4], f32)
        loads.append(nc.gpsimd.dma_start(bq_sb[:], bq2[:]))
        bo_sb = const_pool.tile([128, 4], f32)
        loads.append(nc.gpsimd.dma_start(bo_sb[:], bo2[:]))
        pad_sb = const_pool.tile([128, MTA], f32)
        loads.append(nc.gpsimd.dma_start(pad_sb[:], pad[:]))
        pad8_sb = const_pool.tile([128, MTA, H], f32)
        loads.append(nc.gpsimd.dma_start(pad8_sb[:], pad8[:]))
        ident_sb = const_pool.tile([128, 128], bf16)
        loads.append(nc.gpsimd.dma_start(ident_sb[:], ident[:]))
        wo_sb = persist.tile([128, 4, D], bf16)

        kT_sb = persist.tile([128, 4, N], bf16)
        qT_sb = persist.tile([128, 4, NS], bf16)
        v_sb = persist.tile([128, MTA, H, C + 1], bf16)
        OT_sb = persist.tile([128, 4, NS], bf16)
        # normalized oT per pass: [n, nc4, dh-block, head-in-block, c]
        OTn_sb = persist.tile([128, 4, 2, 2, C], bf16)
        outT_sb = persist.tile([128, 4, NS], bf16)

        # ---- Phase A (part 1): DMAs + projections needed by pass 0 ----
        xw_pool = ctx.enter_context(tc.tile_pool(name="xw", bufs=1))
        wq_sb = xw_pool.tile([128, 4, D], bf16, tag="w")
        wk_sb = xw_pool.tile([128, 4, D], bf16, tag="w2")
        wv_sb = xw_pool.tile([128, 4, D], bf16, tag="w3")
        xtq_sb = xw_pool.tile([128, 4, NS], bf16, tag="xq")
        xtk_sb = xw_pool.tile([128, 4, N], bf16, tag="xk")
        xtv_sb = xw_pool.tile([128, 4, N], bf16, tag="xv")
        # q/k path on SP, v path on the Pool queue: parallel dispatch halves
        # the dispatch-bound lead-in.
        nc.sync.dma_start(wq_sb[:, :, 0:256],
                          wq[:, 0:256].rearrange("(c p) d -> p c d", p=128))
        nc.sync.dma_start(xtq_sb[:, 0:2, :],
                          xtq[0:256].rearrange("(c p) n -> p c n", p=128))
        nc.sync.dma_start(xtq_sb[:, 2:4, :],
                          xtq[256:512].rearrange("(c p) n -> p c n", p=128))
        nc.sync.dma_start(wk_sb[:, :, 0:256],
                          wk[:, 0:256].rearrange("(c p) d -> p c d", p=128))
        nc.gpsimd.dma_start(wv_sb[:],
                            wv[:].rearrange("(c p) d -> p c d", p=128))
        for mb in range(4):
            sl = slice(mb * 512, (mb + 1) * 512)
            nc.sync.dma_start(
                xtk_sb[:, :, sl],
                xtk[:, sl].rearrange("(kc p) m -> p kc m", p=128))
            nc.gpsimd.dma_start(
                xtv_sb[:, :, sl],
                xtv[:, sl].rearrange("(kc p) m -> p kc m", p=128))

        nc.sync.dma_start(wq_sb[:, :, 256:512],
                          wq[:, 256:512].rearrange("(c p) d -> p c d", p=128))
        nc.sync.dma_start(wk_sb[:, :, 256:512],
                          wk[:, 256:512].rearrange("(c p) d -> p c d", p=128))

        # E-plane tiles: quarter planes [128, 4, 2, NS] bf16 per fetch
        ep_pool = ctx.enter_context(tc.tile_pool(name="ep", bufs=_B("EP", 6)))
        pT_pool = ctx.enter_context(tc.tile_pool(name="pT", bufs=_B("PT", 5)))
        p2_pool = ctx.enter_context(tc.tile_pool(name="p2", bufs=_B("P2", 5)))
        EH = 4
        ep_tiles = {}

        def fetch_e(hp, q, eng=None):
            h0 = q * EH
            hn = min(EH, MTA - h0)
            t = ep_pool.tile([128, EH, 2, NS], bf16, tag="ep")
            (eng or nc.gpsimd).dma_start(t[:, 0:hn, :, :],
                                         epl[hp, :, h0:h0 + hn, :, :])
            ep_tiles[(hp, q)] = t

        fetch_e(0, 0)
        fetch_e(1, 0)
        fetch_e(0, 1)
        fetch_e(1, 1)
        fetch_e(0, 2)
        fetch_e(1, 2)

        vones = [nc.vector.tensor_copy(
            v_sb[:, :, :, C : C + 1],
            pad8_sb[:, :, :].rearrange("p m (h o) -> p m h o", o=1))]

        with tc.tile_pool(name="psA", bufs=_B("PSA", 8), space="PSUM") as psA:
            # qT[dh, n] = (Wq/8).T @ xT_q  (+ bq/8 per-partition), heads 0-3
            for j in range(2):
                ps = psA.tile([128, NS], f32, tag="psA")
                for kc in range(4):
                    mm(ps[:], wq_sb[:, kc, j * 128:(j + 1) * 128],
                       xtq_sb[:, kc, :], start=(kc == 0), stop=(kc == 3))
                nc.scalar.activation(qT_sb[:, j, :], ps[:], AFT.Identity,
                                     bias=bq_sb[:, j:j + 1])

            # kT[dh, m] = Wk.T @ xT_k, heads 0-3 (k bias drops in softmax)
            for mb in range(4):
                for j in range(2):
                    ps = psA.tile([128, NS], f32, tag="psA")
                    for kc in range(4):
                        mm(ps[:], wk_sb[:, kc, j * 128:(j + 1) * 128],
                           xtk_sb[:, kc, mb * 512:(mb + 1) * 512],
                           start=(kc == 0), stop=(kc == 3))
                    if (mb + j) % 2 == 0:
                        nc.scalar.copy(kT_sb[:, j, mb * 512:(mb + 1) * 512],
                                       ps[:])
                    else:
                        nc.vector.tensor_copy(
                            kT_sb[:, j, mb * 512:(mb + 1) * 512], ps[:])

            # v[m, c] = xT_v.T @ Wv, padded key rows zeroed; evacuations
            # split DVE / ACT(scale) so they don't serialize the lead-in
            for mi, mt in enumerate(active):
                ps = psA.tile([128, D], f32, tag="psA")
                for kc in range(4):
                    mm(ps[:], xtv_sb[:, kc, mt * 128:(mt + 1) * 128],
                       wv_sb[:, kc, :], start=(kc == 0), stop=(kc == 3))
                if mi % 2 == 0:
                    nc.vector.tensor_scalar(
                        v_sb[:, mi, :, 0:C],
                        ps[:].rearrange("p (h c) -> p h c", h=H),
                        pad_sb[:, mi:mi + 1], None, ALU.mult)
                else:
                    nc.scalar.activation(
                        v_sb[:, mi, :, 0:C],
                        ps[:].rearrange("p (h c) -> p h c", h=H),
                        AFT.Identity, scale=pad_sb[:, mi:mi + 1])

        # PSUM pools for phase B (psA released its banks above)
        # psO-tag ring: 4 slots of 1 bank each.  Holds in turn: the o
        # accumulators ([128, 4, 65] f32: 4 heads + rowsum col per n-chunk),
        # the deferred j23 k projections ([128, 512] f32), and the output
        # projection tiles.  psS tiles are [128, 2, NS] (2 banks).
        small_pool = ctx.enter_context(tc.tile_pool(name="small", bufs=4))
        psO = ctx.enter_context(tc.tile_pool(name="psO", bufs=4, space="PSUM"))
        psS = ctx.enter_context(tc.tile_pool(name="psS", bufs=_B("PSS", 2), space="PSUM"))

        # ---- Phase B: attention, two passes of 4 heads (2 head pairs) ----
        def attn_pass(p, hooks={}):
            o_ps = [psO.tile([128, 4, C + 1], f32, tag="psO",
                             name=f"o_ps{p}_{i}") for i in range(4)]

            def emit_o(p2, mi, hpl):
                for j in range(2):
                    h = 4 * p + 2 * hpl + j
                    hl = 2 * hpl + j
                    for nc4 in range(4):
                        mm(o_ps[nc4][:, hl, :],
                           p2[:, j, nc4 * 128:(nc4 + 1) * 128],
                           v_sb[:, mi, h, :],
                           start=(mi == 0 and hl == 0),
                           stop=(mi == MTA - 1 and hl == 3),
                           skip_group_check=True)

            pending = []
            for mi in range(MTA):
                for fn in hooks.get(mi, ()):
                    fn()
                for hpl in range(2):
                    hp = 2 * p + hpl
                    if p == 0 and hpl == 0 and mi in (6, 8, 10, 12):
                        nq = {6: [(0, 3), (1, 3)], 8: [(2, 0), (3, 0)],
                              10: [(2, 1), (3, 1)], 12: [(2, 2), (3, 2)]}[mi]
                        for a, b in nq:
                            fetch_e(a, b)
                    if p == 1 and hpl == 0 and mi == 4:
                        fetch_e(2, 3)
                        fetch_e(3, 3)
                    s_ps = psS.tile([128, 2, NS], f32, tag="psS")
                    for j in range(2):
                        h = 4 * p + 2 * hpl + j
                        hj, ho = h // 2, (h % 2) * 64
                        mm(s_ps[:, j, :],
                           kT_sb[ho:ho + 64, hj, active[mi] * 128:active[mi] * 128 + 128],
                           qT_sb[ho:ho + 64, hj, :], start=True, stop=True)
                    pT = pT_pool.tile([128, 2, NS], bf16, tag="pT")
                    nc.scalar.activation(pT[:], s_ps[:], AFT.Exp)
                    p2 = p2_pool.tile([128, 2, NS], bf16, tag="p2")
                    # all-bf16 tensor_tensor: DVE 2x_1p fast mode (~593ns)
                    nc.vector.tensor_tensor(
                        p2[:], pT[:],
                        ep_tiles[(hp, mi // EH)][:, mi % EH, :, :], ALU.mult)
                    # transposed o accumulation (emit_o), two groups late
                    # so the in-order PE queue never blocks on the live
                    # exp+mult chain or the boundary normalize WAR.
                    pending.append((p2, mi, hpl))
                    if len(pending) > 2:
                        emit_o(*pending.pop(0))
            for args in pending:
                emit_o(*args)
            return o_ps

        def normalize(p, o_ps, halves=(0, 1), nc4s=(0, 1, 2, 3)):
            # oT[n, c] * (1/rowsum[n]): per-partition reciprocal (DVE) +
            # scalar multiply, split DVE/ACT (ACT is idle at pass ends;
            # tensor_scalar divide fails the walrus ISA check).
            for half in halves:
                for k, hl in enumerate((2 * half, 2 * half + 1)):
                    for nc4 in nc4s:
                        rec = small_pool.tile([128, 1], f32, tag="rec",
                                              name=f"rc{p}_{hl}_{nc4}")
                        nc.vector.reciprocal(rec[:], o_ps[nc4][:, hl, C:C + 1])
                        if (nc4 + k) % 2 == 0:
                            nc.vector.tensor_scalar(
                                OTn_sb[:, nc4, half, hl % 2, :],
                                o_ps[nc4][:, hl, 0:C],
                                rec[:], None, ALU.mult)
                        else:
                            nc.scalar.activation(
                                OTn_sb[:, nc4, half, hl % 2, :],
                                o_ps[nc4][:, hl, 0:C],
                                AFT.Identity, scale=rec[:])

        def transposes(p, b, eng):
            # [n, dh-block] -> [dh-block, n] via PE transpose, then one
            # 512-wide evacuation copy into OT_sb[:, 2p+b, :].
            tp = psS.tile([128, 4, 128], bf16, tag="psS", name=f"tp{p}_{b}")
            for nc4 in range(4):
                nc.tensor.transpose(tp[:, nc4, :], OTn_sb[:, nc4, b, :, :],
                                    ident_sb[:])
            if eng is nc.scalar:
                eng.copy(OT_sb[:, 2 * p + b, :],
                         tp[:].rearrange("p a n -> p (a n)"))
            else:
                eng.tensor_copy(OT_sb[:, 2 * p + b, :],
                                tp[:].rearrange("p a n -> p (a n)"))

        # deferred projections for heads 4-7 (j-blocks 2,3)
        def proj_j23_k_hook(mb):
            ps = psS.tile([128, 2, NS], f32, tag="psS", name=f"kh{mb}")
            for j in (2, 3):
                for kc in range(4):
                    mm(ps[:, j - 2, :], wk_sb[:, kc, j * 128:(j + 1) * 128],
                       xtk_sb[:, kc, mb * 512:(mb + 1) * 512],
                       start=(kc == 0), stop=(kc == 3))
            nc.vector.tensor_copy(
                kT_sb[:, 2, mb * 512:(mb + 1) * 512], ps[:, 0, :])
            nc.vector.tensor_copy(
                kT_sb[:, 3, mb * 512:(mb + 1) * 512], ps[:, 1, :])

        def proj_j23_k_psS(mb):
            ps = psS.tile([128, 2, NS], f32, tag="psS", name=f"kp{mb}")
            for j in (2, 3):
                for kc in range(4):
                    mm(ps[:, j - 2, :], wk_sb[:, kc, j * 128:(j + 1) * 128],
                       xtk_sb[:, kc, mb * 512:(mb + 1) * 512],
                       start=(kc == 0), stop=(kc == 3))
            nc.scalar.copy(kT_sb[:, 2, mb * 512:(mb + 1) * 512], ps[:, 0, :])
            nc.vector.tensor_copy(
                kT_sb[:, 3, mb * 512:(mb + 1) * 512], ps[:, 1, :])

        def proj_j23_q_psS():
            ps = psS.tile([128, 2, NS], f32, tag="psS", name="qp23")
            for j in (2, 3):
                for kc in range(4):
                    mm(ps[:, j - 2, :], wq_sb[:, kc, j * 128:(j + 1) * 128],
                       xtq_sb[:, kc, :], start=(kc == 0), stop=(kc == 3))
            for j in (2, 3):
                nc.scalar.activation(qT_sb[:, j, :], ps[:, j - 2, :],
                                     AFT.Identity, bias=bq_sb[:, j:j + 1])

        def proj_j23_k_psO(mb):
            for j in (2, 3):
                ps = psO.tile([128, NS], f32, tag="psO", name=f"kp{mb}_{j}")
                for kc in range(4):
                    mm(ps[:], wk_sb[:, kc, j * 128:(j + 1) * 128],
                       xtk_sb[:, kc, mb * 512:(mb + 1) * 512],
                       start=(kc == 0), stop=(kc == 3))
                if j == 2:
                    nc.scalar.copy(
                        kT_sb[:, j, mb * 512:(mb + 1) * 512], ps[:])
                else:
                    nc.vector.tensor_copy(
                        kT_sb[:, j, mb * 512:(mb + 1) * 512], ps[:])

        o_ps0 = attn_pass(0)
        proj_j23_k_psS(0)
        proj_j23_q_psS()
        proj_j23_k_psS(1)
        # both normalize halves must finish reading the o banks before any
        # psO-ring reuse: each bank holds all four heads of the pass.
        normalize(0, o_ps0, (0,))
        normalize(0, o_ps0, (1,))
        transposes(0, 0, nc.vector)
        transposes(0, 1, nc.scalar)
        proj_j23_k_psO(2)
        proj_j23_k_psO(3)
        for c in range(4):
            nc.sync.dma_start(wo_sb[:, c, :], wo[c * 128:(c + 1) * 128, :])

        o_ps1 = attn_pass(1)

        # ---- Phase C: output projection, pipelined with pass-1 tail ----
        def outproj_partial(jt):
            ps = psO.tile([128, NS], f32, tag="psO", name=f"oc{jt}")
            for g in (0, 1, 2):
                mm(ps[:], wo_sb[:, g, jt * 128:(jt + 1) * 128],
                   OT_sb[:, g, :], start=(g == 0), stop=False)
            return ps

        def outproj_finish(jt, ps=None):
            if ps is None:
                ps = psO.tile([128, NS], f32, tag="psO", name=f"oc{jt}")
                for g in (0, 1, 2):
                    mm(ps[:], wo_sb[:, g, jt * 128:(jt + 1) * 128],
                       OT_sb[:, g, :], start=(g == 0), stop=False)
            mm(ps[:], wo_sb[:, 3, jt * 128:(jt + 1) * 128],
               OT_sb[:, 3, :], start=False, stop=True)
            if jt % 2 == 0:
                nc.scalar.activation(outT_sb[:, jt, :], ps[:], AFT.Identity,
                                     bias=bo_sb[:, jt:jt + 1])
            else:
                nc.vector.tensor_scalar(outT_sb[:, jt, :], ps[:],
                                        bo_sb[:, jt:jt + 1], None, ALU.add)
            nc.sync.dma_start(outT[jt * 128:(jt + 1) * 128, :],
                              outT_sb[:, jt, :])

        normalize(1, o_ps1, (0,))
        normalize(1, o_ps1, (1,))
        transposes(1, 0, nc.vector)
        pc0 = outproj_partial(0)
        pc1 = outproj_partial(1)
        transposes(1, 1, nc.scalar)
        outproj_finish(0, pc0)
        outproj_finish(1, pc1)
        outproj_finish(2)
        outproj_finish(3)


# every TPB instruction encoding in this walrus build tolerates only a
# single semaphore wait -- split extras regardless of opcode
_NO_SPLIT_TYPES = {"InstEventSemaphore"}


def _split_matmul_waits(nc, mybir):
    """Several engine instruction encodings tolerate only one semaphore
    wait; move extra waits onto standalone single-wait EventSemaphore
    instructions inserted right before them on the same engine queue."""
    import bass_rust

    n = 0
    for bb in nc.m.functions[0].blocks:
        insts = list(bb.instructions)
        out = []
        changed = False
        for i in insts:
            si = i.sync_info
            if (type(i).__name__ not in _NO_SPLIT_TYPES and si is not None
                    and len(si.on_wait) > 1):
                w = list(si.on_wait)
                for wx in w[:-1]:
                    ev = mybir.InstEventSemaphore(name=f"mmw_{n}_{i.name}",
                                                  ins=[], outs=[])
                    ev.engine = i.engine
                    ev.sync_info = bass_rust.SyncInfo(on_wait=[wx],
                                                      on_update=[])
                    out.append(ev)
                    n += 1
                si.on_wait = [w[-1]]
                changed = True
            out.append(i)
        if changed:
            bb.instructions = out


def _host_prep(inputs):
    x_q = np.asarray(inputs["x_q"], np.float32)
    x_k = np.asarray(inputs["x_k"], np.float32)
    x_v = np.asarray(inputs["x_v"], np.float32)
    attn_mask = np.asarray(inputs["attn_mask"]).astype(bool)
    kpm = np.asarray(inputs["key_padding_mask"]).astype(bool)
    Wq = np.asarray(inputs["Wq"], np.float32)
    Wk = np.asarray(inputs["Wk"], np.float32)
    Wv = np.asarray(inputs["Wv"], np.float32)
    Wo = np.asarray(inputs["Wo"], np.float32)
    bq = np.asarray(inputs["bq"], np.float32)
    bv = np.asarray(inputs["bv"], np.float32)
    bo = np.asarray(inputs["bo"], np.float32)
    mw = np.asarray(inputs["mask_weight"], np.float64)

    # c[h,r] = softmax(mask_weight[h,:R]) * mask_weight[h,R]
    e = np.exp(mw[:, :R] - mw[:, :R].max(axis=1, keepdims=True))
    w = e / e.sum(axis=1, keepdims=True)
    c = (w * mw[:, R:R + 1]).astype(np.float32)          # [H, R]

    # active key tiles (at least one unpadded key) -- shared across batch
    tile_padded = kpm.reshape(B, MT, 128).all(axis=2)    # [B, MT]
    active = [mt for mt in range(MT) if not tile_padded[:, mt].all()]
    MTA = len(active)

    scale = np.float32(1.0 / np.sqrt(C))
    wq_s = (Wq * scale).astype(np.float32)
    bq_s = (bq * scale).astype(np.float32)
    bo_p = (bv @ Wo + bo).astype(np.float32)

    bq2 = np.ascontiguousarray(bq_s.reshape(4, 128).T)
    bo2 = np.ascontiguousarray(bo_p.reshape(4, 128).T)

    bf = ml_dtypes.bfloat16
    common = dict(wq=wq_s.astype(bf), wk=Wk.astype(bf), wv=Wv.astype(bf),
                  wo=Wo.astype(bf), bq2=bq2, bo2=bo2,
                  ident=np.eye(128, dtype=bf))

    emul = np.exp(-c)                                    # [H, R] in (0,1]
    in_maps = []
    for core in range(NCORES):
        b, ns = core // 4, core % 4
        n0 = ns * NS
        pad01 = (~kpm[b]).astype(np.float32)             # [N]
        pad2 = np.ascontiguousarray(
            pad01.reshape(MT, 128).T[:, active])         # [128, MTA]
        pad8 = np.ascontiguousarray(np.repeat(pad2[:, :, None], H, axis=2))
        inv = attn_mask[b, :, n0:n0 + NS, :]             # [R, NS, N]
        ep = np.empty((H // 2, 128, MTA, 2, NS), bf)
        for mi, mt in enumerate(active):
            invt = inv[:, :, mt * 128:(mt + 1) * 128]    # [R, NS, 128]
            bias = np.einsum('hr,rnm->hmn', c, invt.astype(np.float32))
            ep[:, :, mi] = np.exp(-bias).astype(bf).reshape(
                H // 2, 2, 128, NS).transpose(0, 2, 1, 3)
        ep = np.ascontiguousarray(ep)
        m = dict(common)
        m["xtq"] = np.ascontiguousarray(x_q[n0:n0 + NS, b, :].T).astype(bf)
        m["xtk"] = np.ascontiguousarray(x_k[:, b, :].T).astype(bf)
        m["xtv"] = np.ascontiguousarray(x_v[:, b, :].T).astype(bf)
        m["epl"] = ep
        m["pad"] = pad2
        m["pad8"] = pad8
        in_maps.append(m)
    return in_maps, active


def kernel(**inputs) -> np.ndarray:
    from concourse.bass_utils import run_bass_kernel_spmd

    in_maps, active = _host_prep(inputs)
    key = tuple(active)
    if key not in _cache:
        _cache[key] = _build_program(active)
        _cache["nc"] = _cache[key]
    nc = _cache[key]

    res = run_bass_kernel_spmd(nc, in_maps, list(range(NCORES)))

    out = np.empty((N, B, D), np.float32)
    for core in range(NCORES):
        b, ns = core // 4, core % 4
        n0 = ns * NS
        out[n0:n0 + NS, b, :] = res.results[core]["outT"].T.astype(np.float32)
    return out


# revision 6
# speedup vs baseline: 1.0236x; 1.0032x over previous
"""Bass/Trainium2 kernel for DynamicMultiheadAttention (sparse_attention).

v2: attention@V runs in TRANSPOSED orientation — oT[n, c] = p2.T @ v with
p2 (scores) as the PE stationary and v ([128, 65] incl. ones column) as the
moving tensor.  PE matmul cost is output-free-size cycles, so the o-path
drops from 120x[65,512] (213ns) to 480x[128,65] (27ns): ~-12.6us PE.
The softmax rowsum lands as PSUM column 64 per (n-partition, head), so
normalization is a per-partition tensor_scalar DIVIDE (no reciprocal
broadcasts, no ones2 matmuls, no ACT copies).  The normalized oT is
PE-transposed back ([n,dh] -> [dh,n], 16x 53ns) to feed the output
projection, which is unchanged.

Sharding: 8 cores = (batch b in {0,1}) x (query-slice of 512 rows).
Each core computes all 8 heads for its (b, n-slice); scores sT[m, n]
with keys m on partitions.

The relative-mask bias is applied multiplicatively after the exp:
  exp(s + rel) = exp(s) * E,  E[h,m,n] = exp(-sum_r c[h,r]*attn_mask)
E is precomputed on the host as bf16 planes; the all-16-bit
tensor_tensor multiply runs in the DVE 2x_1p fast mode (~593ns per
two-head tile).

Fully-padded key tiles are skipped at program-build time; partially
padded tiles are handled by zeroing the affected rows of V and of the
appended ones-column.  The k bias is softmax-invariant and dropped; the
v bias folds into the output bias: bo' = bv @ Wo + bo.

Every TPB instruction encoding in this walrus build tolerates only ONE
semaphore wait; a post-pass (_split_matmul_waits) moves extra waits onto
standalone single-wait EventSemaphore instructions.
"""

import numpy as np
import ml_dtypes
import os

def _B(name, default):
    return int(os.environ.get("KB_" + name, default))

N, B, D = 2048, 2, 512
H, R = 8, 3
C = D // H          # 64
NS = N // 4         # 512 query rows per core
NCORES = 8
MT = N // 128       # 16 key tiles

_cache = {}


def _build_program(active, reps=1):
    import concourse.bass as bass
    import concourse.mybir as mybir
    import concourse.tile as tile
    from contextlib import ExitStack

    f32 = mybir.dt.float32
    f32r = mybir.dt.float32r
    bf16 = mybir.dt.bfloat16
    u8 = mybir.dt.uint8
    AFT = mybir.ActivationFunctionType
    ALU = mybir.AluOpType

    MTA = len(active)

    nc = bass.Bass()

    xtq = nc.declare_dram_parameter("xtq", [D, NS], bf16, isOutput=False)
    xtk = nc.declare_dram_parameter("xtk", [D, N], bf16, isOutput=False)
    xtv = nc.declare_dram_parameter("xtv", [D, N], bf16, isOutput=False)
    # E planes, partition-major: [hp, p(=m%128), mi, j(head in pair), n]
    epl = nc.declare_dram_parameter("epl", [H // 2, 128, MTA, 2, NS], bf16,
                                    isOutput=False)
    wq = nc.declare_dram_parameter("wq", [D, D], bf16, isOutput=False)
    wk = nc.declare_dram_parameter("wk", [D, D], bf16, isOutput=False)
    wv = nc.declare_dram_parameter("wv", [D, D], bf16, isOutput=False)
    wo = nc.declare_dram_parameter("wo", [D, D], bf16, isOutput=False)
    bq2 = nc.declare_dram_parameter("bq2", [128, 4], f32, isOutput=False)
    bo2 = nc.declare_dram_parameter("bo2", [128, 4], f32, isOutput=False)
    # per-active-tile pad multiplier planes (zero padded key rows of V)
    pad = nc.declare_dram_parameter("pad", [128, MTA], f32, isOutput=False)
    pad8 = nc.declare_dram_parameter("pad8", [128, MTA, H], f32, isOutput=False)
    ident = nc.declare_dram_parameter("ident", [128, 128], bf16, isOutput=False)
    outT = nc.declare_dram_parameter("outT", [D, NS], bf16, isOutput=True)

    with tile.TileContext(nc) as tc, ExitStack() as ctx:
        mm = nc.tensor.matmul
        _run_once(nc, tc, ctx, mm, tile, mybir, f32, f32r, bf16, u8,
                  AFT, ALU, active, xtq, xtk, xtv, epl, wq, wk, wv, wo,
                  bq2, bo2, pad, pad8, ident, outT)

    _split_matmul_waits(nc, mybir)
    return nc


def _run_once(nc, tc, ctx, mm, tile, mybir, f32, f32r, bf16, u8, AFT, ALU,
              active, xtq, xtk, xtv, epl, wq, wk, wv, wo, bq2, bo2,
              pad, pad8, ident, outT):
    from contextlib import ExitStack
    MTA = len(active)
    with ExitStack() as ctx:
        const_pool = ctx.enter_context(tc.tile_pool(name="const", bufs=1))
        persist = ctx.enter_context(tc.tile_pool(name="persist", bufs=1))

        # constants ride the Pool queue: SP's 650ns-per-DMA dispatch rate is
        # the lead-in bottleneck, so it is reserved for the q/k-path inputs
        loads = []
        bq_sb = const_pool.tile([128, 4], f32)
        loads.append(nc.gpsimd.dma_start(bq_sb[:], bq2[:]))
        bo_sb = const_pool.tile([128, 4], f32)
        loads.append(nc.gpsimd.dma_start(bo_sb[:], bo2[:]))
        pad_sb = const_pool.tile([128, MTA], f32)
        loads.append(nc.gpsimd.dma_start(pad_sb[:], pad[:]))
        pad8_sb = const_pool.tile([128, MTA, H], f32)
        loads.append(nc.gpsimd.dma_start(pad8_sb[:], pad8[:]))
        ident_sb = const_pool.tile([128, 128], bf16)
        loads.append(nc.gpsimd.dma_start(ident_sb[:], ident[:]))
        wo_sb = persist.tile([128, 4, D], bf16)

        kT_sb = persist.tile([128, 4, N], bf16)
        qT_sb = persist.tile([128, 4, NS], bf16)
        v_sb = persist.tile([128, MTA, H, C + 1], bf16)
        OT_sb = persist.tile([128, 4, NS], bf16)
        # normalized oT per pass: [n, nc4, dh-block, head-in-block, c]
        OTn_sb = persist.tile([128, 4, 2, 2, C], bf16)
        outT_sb = persist.tile([128, 4, NS], bf16)

        # ---- Phase A (part 1): DMAs + projections needed by pass 0 ----
        xw_pool = ctx.enter_context(tc.tile_pool(name="xw", bufs=1))
        wq_sb = xw_pool.tile([128, 4, D], bf16, tag="w")
        wk_sb = xw_pool.tile([128, 4, D], bf16, tag="w2")
        wv_sb = xw_pool.tile([128, 4, D], bf16, tag="w3")
        xtq_sb = xw_pool.tile([128, 4, NS], bf16, tag="xq")
        xtk_sb = xw_pool.tile([128, 4, N], bf16, tag="xk")
        xtv_sb = xw_pool.tile([128, 4, N], bf16, tag="xv")
        # q/k path on SP, v path on the Pool queue: parallel dispatch halves
        # the dispatch-bound lead-in.
        nc.sync.dma_start(wq_sb[:, :, 0:256],
                          wq[:, 0:256].rearrange("(c p) d -> p c d", p=128))
        nc.sync.dma_start(xtq_sb[:, 0:2, :],
                          xtq[0:256].rearrange("(c p) n -> p c n", p=128))
        nc.sync.dma_start(xtq_sb[:, 2:4, :],
                          xtq[256:512].rearrange("(c p) n -> p c n", p=128))
        nc.sync.dma_start(wk_sb[:, :, 0:256],
                          wk[:, 0:256].rearrange("(c p) d -> p c d", p=128))
        nc.gpsimd.dma_start(wv_sb[:],
                            wv[:].rearrange("(c p) d -> p c d", p=128))
        for mb in range(4):
            sl = slice(mb * 512, (mb + 1) * 512)
            nc.sync.dma_start(
                xtk_sb[:, :, sl],
                xtk[:, sl].rearrange("(kc p) m -> p kc m", p=128))
            nc.gpsimd.dma_start(
                xtv_sb[:, :, sl],
                xtv[:, sl].rearrange("(kc p) m -> p kc m", p=128))

        nc.sync.dma_start(wq_sb[:, :, 256:512],
                          wq[:, 256:512].rearrange("(c p) d -> p c d", p=128))
        nc.sync.dma_start(wk_sb[:, :, 256:512],
                          wk[:, 256:512].rearrange("(c p) d -> p c d", p=128))

        # E-plane tiles: quarter planes [128, 4, 2, NS] bf16 per fetch
        ep_pool = ctx.enter_context(tc.tile_pool(name="ep", bufs=_B("EP", 6)))
        pT_pool = ctx.enter_context(tc.tile_pool(name="pT", bufs=_B("PT", 5)))
        p2_pool = ctx.enter_context(tc.tile_pool(name="p2", bufs=_B("P2", 5)))
        EH = 4
        ep_tiles = {}

        def fetch_e(hp, q, eng=None):
            h0 = q * EH
            hn = min(EH, MTA - h0)
            t = ep_pool.tile([128, EH, 2, NS], bf16, tag="ep")
            (eng or nc.gpsimd).dma_start(t[:, 0:hn, :, :],
                                         epl[hp, :, h0:h0 + hn, :, :])
            ep_tiles[(hp, q)] = t

        fetch_e(0, 0)
        fetch_e(1, 0)
        fetch_e(0, 1)
        fetch_e(1, 1)
        fetch_e(0, 2)
        fetch_e(1, 2)

        vones = [nc.vector.tensor_copy(
            v_sb[:, :, :, C : C + 1],
            pad8_sb[:, :, :].rearrange("p m (h o) -> p m h o", o=1))]

        with tc.tile_pool(name="psA", bufs=_B("PSA", 8), space="PSUM") as psA:
            # qT[dh, n] = (Wq/8).T @ xT_q  (+ bq/8 per-partition), heads 0-3
            for j in range(2):
                ps = psA.tile([128, NS], f32, tag="psA")
                for kc in range(4):
                    mm(ps[:], wq_sb[:, kc, j * 128:(j + 1) * 128],
                       xtq_sb[:, kc, :], start=(kc == 0), stop=(kc == 3))
                nc.scalar.activation(qT_sb[:, j, :], ps[:], AFT.Identity,
                                     bias=bq_sb[:, j:j + 1])

            # kT[dh, m] = Wk.T @ xT_k, heads 0-3 (k bias drops in softmax)
            for mb in range(4):
                for j in range(2):
                    ps = psA.tile([128, NS], f32, tag="psA")
                    for kc in range(4):
                        mm(ps[:], wk_sb[:, kc, j * 128:(j + 1) * 128],
                           xtk_sb[:, kc, mb * 512:(mb + 1) * 512],
                           start=(kc == 0), stop=(kc == 3))
                    if (mb + j) % 2 == 0:
                        nc.scalar.copy(kT_sb[:, j, mb * 512:(mb + 1) * 512],
                                       ps[:])
                    else:
                        nc.vector.tensor_copy(
                            kT_sb[:, j, mb * 512:(mb + 1) * 512], ps[:])

            # v[m, c] = xT_v.T @ Wv, padded key rows zeroed; evacuations
            # split DVE / ACT(scale) so they don't serialize the lead-in
            for mi, mt in enumerate(active):
                ps = psA.tile([128, D], f32, tag="psA")
                for kc in range(4):
                    mm(ps[:], xtv_sb[:, kc, mt * 128:(mt + 1) * 128],
                       wv_sb[:, kc, :], start=(kc == 0), stop=(kc == 3))
                if mi % 2 == 0:
                    nc.vector.tensor_scalar(
                        v_sb[:, mi, :, 0:C],
                        ps[:].rearrange("p (h c) -> p h c", h=H),
                        pad_sb[:, mi:mi + 1], None, ALU.mult)
                else:
                    nc.scalar.activation(
                        v_sb[:, mi, :, 0:C],
                        ps[:].rearrange("p (h c) -> p h c", h=H),
                        AFT.Identity, scale=pad_sb[:, mi:mi + 1])

        # PSUM pools for phase B (psA released its banks above)
        # psO-tag ring: 4 slots of 1 bank each.  Holds in turn: the o
        # accumulators ([128, 4, 65] f32: 4 heads + rowsum col per n-chunk),
        # the deferred j23 k projections ([128, 512] f32), and the output
        # projection tiles.  psS tiles are [128, 2, NS] (2 banks).
        small_pool = ctx.enter_context(tc.tile_pool(name="small", bufs=4))
        psO = ctx.enter_context(tc.tile_pool(name="psO", bufs=4, space="PSUM"))
        psS = ctx.enter_context(tc.tile_pool(name="psS", bufs=_B("PSS", 2), space="PSUM"))

        # ---- Phase B: attention, two passes of 4 heads (2 head pairs) ----
        def attn_pass(p, hooks={}):
            o_ps = [psO.tile([128, 4, C + 1], f32, tag="psO",
                             name=f"o_ps{p}_{i}") for i in range(4)]

            def emit_o(p2, mi, hpl):
                for j in range(2):
                    h = 4 * p + 2 * hpl + j
                    hl = 2 * hpl + j
                    for nc4 in range(4):
                        mm(o_ps[nc4][:, hl, :],
                           p2[:, j, nc4 * 128:(nc4 + 1) * 128],
                           v_sb[:, mi, h, :],
                           start=(mi == 0 and hl == 0),
                           stop=(mi == MTA - 1 and hl == 3),
                           skip_group_check=True)

            pending = []
            for mi in range(MTA):
                for fn in hooks.get(mi, ()):
                    fn()
                for hpl in range(2):
                    hp = 2 * p + hpl
                    if p == 0 and hpl == 0 and mi in (6, 8, 10, 12):
                        nq = {6: [(0, 3), (1, 3)], 8: [(2, 0), (3, 0)],
                              10: [(2, 1), (3, 1)], 12: [(2, 2), (3, 2)]}[mi]
                        for a, b in nq:
                            fetch_e(a, b)
                    if p == 1 and hpl == 0 and mi == 4:
                        fetch_e(2, 3)
                        fetch_e(3, 3)
                    s_ps = psS.tile([128, 2, NS], f32, tag="psS")
                    for j in range(2):
                        h = 4 * p + 2 * hpl + j
                        hj, ho = h // 2, (h % 2) * 64
                        mm(s_ps[:, j, :],
                           kT_sb[ho:ho + 64, hj, active[mi] * 128:active[mi] * 128 + 128],
                           qT_sb[ho:ho + 64, hj, :], start=True, stop=True)
                    pT = pT_pool.tile([128, 2, NS], bf16, tag="pT")
                    nc.scalar.activation(pT[:], s_ps[:], AFT.Exp)
                    p2 = p2_pool.tile([128, 2, NS], bf16, tag="p2")
                    # all-bf16 tensor_tensor: DVE 2x_1p fast mode (~593ns)
                    nc.vector.tensor_tensor(
                        p2[:], pT[:],
                        ep_tiles[(hp, mi // EH)][:, mi % EH, :, :], ALU.mult)
                    # transposed o accumulation (emit_o), two groups late
                    # so the in-order PE queue never blocks on the live
                    # exp+mult chain or the boundary normalize WAR.
                    pending.append((p2, mi, hpl))
                    if len(pending) > 2:
                        emit_o(*pending.pop(0))
            for args in pending:
                emit_o(*args)
            return o_ps

        def normalize(p, o_ps, halves=(0, 1), nc4s=(0, 1, 2, 3)):
            # oT[n, c] * (1/rowsum[n]): per-partition reciprocal (DVE) +
            # scalar multiply, split DVE/ACT (ACT is idle at pass ends;
            # tensor_scalar divide fails the walrus ISA check).
            for half in halves:
                for k, hl in enumerate((2 * half, 2 * half + 1)):
                    for nc4 in nc4s:
                        rec = small_pool.tile([128, 1], f32, tag="rec",
                                              name=f"rc{p}_{hl}_{nc4}")
                        nc.vector.reciprocal(rec[:], o_ps[nc4][:, hl, C:C + 1])
                        if (nc4 + k) % 2 == 0:
                            nc.vector.tensor_scalar(
                                OTn_sb[:, nc4, half, hl % 2, :],
                                o_ps[nc4][:, hl, 0:C],
                                rec[:], None, ALU.mult)
                        else:
                            nc.scalar.activation(
                                OTn_sb[:, nc4, half, hl % 2, :],
                                o_ps[nc4][:, hl, 0:C],
                                AFT.Identity, scale=rec[:])

        def transposes(p, b, eng):
            # [n, dh-block] -> [dh-block, n] via PE transpose, then one
            # 512-wide evacuation copy into OT_sb[:, 2p+b, :].
            tp = psS.tile([128, 4, 128], bf16, tag="psS", name=f"tp{p}_{b}")
            for nc4 in range(4):
                nc.tensor.transpose(tp[:, nc4, :], OTn_sb[:, nc4, b, :, :],
                                    ident_sb[:])
            if eng is nc.scalar:
                eng.copy(OT_sb[:, 2 * p + b, :],
                         tp[:].rearrange("p a n -> p (a n)"))
            else:
                eng.tensor_copy(OT_sb[:, 2 * p + b, :],
                                tp[:].rearrange("p a n -> p (a n)"))

        # deferred projections for heads 4-7 (j-blocks 2,3)
        def proj_j23_k_hook(mb):
            ps = psS.tile([128, 2, NS], f32, tag="psS", name=f"kh{mb}")
            for j in (2, 3):
                for kc in range(4):
                    mm(ps[:, j - 2, :], wk_sb[:, kc, j * 128:(j + 1) * 128],
                       xtk_sb[:, kc, mb * 512:(mb + 1) * 512],
                       start=(kc == 0), stop=(kc == 3))
            nc.vector.tensor_copy(
                kT_sb[:, 2, mb * 512:(mb + 1) * 512], ps[:, 0, :])
            nc.vector.tensor_copy(
                kT_sb[:, 3, mb * 512:(mb + 1) * 512], ps[:, 1, :])

        def proj_j23_k_psS(mb):
            ps = psS.tile([128, 2, NS], f32, tag="psS", name=f"kp{mb}")
            for j in (2, 3):
                for kc in range(4):
                    mm(ps[:, j - 2, :], wk_sb[:, kc, j * 128:(j + 1) * 128],
                       xtk_sb[:, kc, mb * 512:(mb + 1) * 512],
                       start=(kc == 0), stop=(kc == 3))
            nc.scalar.copy(kT_sb[:, 2, mb * 512:(mb + 1) * 512], ps[:, 0, :])
            nc.vector.tensor_copy(
                kT_sb[:, 3, mb * 512:(mb + 1) * 512], ps[:, 1, :])

        def proj_j23_q_psS():
            ps = psS.tile([128, 2, NS], f32, tag="psS", name="qp23")
            for j in (2, 3):
                for kc in range(4):
                    mm(ps[:, j - 2, :], wq_sb[:, kc, j * 128:(j + 1) * 128],
                       xtq_sb[:, kc, :], start=(kc == 0), stop=(kc == 3))
            for j in (2, 3):
                nc.scalar.activation(qT_sb[:, j, :], ps[:, j - 2, :],
                                     AFT.Identity, bias=bq_sb[:, j:j + 1])

        def proj_j23_k_psO(mb):
            for j in (2, 3):
                ps = psO.tile([128, NS], f32, tag="psO", name=f"kp{mb}_{j}")
                for kc in range(4):
                    mm(ps[:], wk_sb[:, kc, j * 128:(j + 1) * 128],
                       xtk_sb[:, kc, mb * 512:(mb + 1) * 512],
                       start=(kc == 0), stop=(kc == 3))
                if j == 2:
                    nc.scalar.copy(
                        kT_sb[:, j, mb * 512:(mb + 1) * 512], ps[:])
                else:
                    nc.vector.tensor_copy(
                        kT_sb[:, j, mb * 512:(mb + 1) * 512], ps[:])

        o_ps0 = attn_pass(0)
        proj_j23_k_psS(0)
        proj_j23_q_psS()
        proj_j23_k_psS(1)
        # both normalize halves must finish reading the o banks before any
        # psO-ring reuse: each bank holds all four heads of the pass.
        normalize(0, o_ps0, (0,))
        normalize(0, o_ps0, (1,))
        transposes(0, 0, nc.vector)
        transposes(0, 1, nc.scalar)
        proj_j23_k_psO(2)
        proj_j23_k_psO(3)
        for c in range(4):
            nc.sync.dma_start(wo_sb[:, c, :], wo[c * 128:(c + 1) * 128, :])

        o_ps1 = attn_pass(1)

        # ---- Phase C: output projection, pipelined with pass-1 tail ----
        def outproj_partial(jt):
            ps = psO.tile([128, NS], f32, tag="psO", name=f"oc{jt}")
            for g in (0, 1, 2):
                mm(ps[:], wo_sb[:, g, jt * 128:(jt + 1) * 128],
                   OT_sb[:, g, :], start=(g == 0), stop=False)
            return ps

        def outproj_finish(jt, ps=None):
            if ps is None:
                ps = psO.tile([128, NS], f32, tag="psO", name=f"oc{jt}")
                for g in (0, 1, 2):
                    mm(ps[:], wo_sb[:, g, jt * 128:(jt + 1) * 128],
                       OT_sb[:, g, :], start=(g == 0), stop=False)
            mm(ps[:], wo_sb[:, 3, jt * 128:(jt + 1) * 128],
               OT_sb[:, 3, :], start=False, stop=True)
            if jt % 2 == 0:
                nc.scalar.activation(outT_sb[:, jt, :], ps[:], AFT.Identity,
                                     bias=bo_sb[:, jt:jt + 1])
            else:
                nc.vector.tensor_scalar(outT_sb[:, jt, :], ps[:],
                                        bo_sb[:, jt:jt + 1], None, ALU.add)
            nc.sync.dma_start(outT[jt * 128:(jt + 1) * 128, :],
                              outT_sb[:, jt, :])

        # tail: g0/g1 out-projection partials run on psS tiles BEFORE
        # normalize(1) (they only need OT hj0/hj1, ready since pass 0);
        # the tail transposes move to the psO ring (post-normalize slots).
        def outproj_pair(pair):
            ps = psS.tile([128, 2, NS], f32, tag="psS", name=f"ocp{pair}")
            for t in range(2):
                jt = 2 * pair + t
                for g in (0, 1):
                    mm(ps[:, t, :], wo_sb[:, g, jt * 128:(jt + 1) * 128],
                       OT_sb[:, g, :], start=(g == 0), stop=False)
            return ps

        def outproj_g(ps, pair, g, stop):
            for t in range(2):
                jt = 2 * pair + t
                mm(ps[:, t, :], wo_sb[:, g, jt * 128:(jt + 1) * 128],
                   OT_sb[:, g, :], start=False, stop=stop)

        def outproj_done(ps, pair):
            for t in range(2):
                jt = 2 * pair + t
                if jt % 2 == 0:
                    nc.scalar.activation(outT_sb[:, jt, :], ps[:, t, :],
                                         AFT.Identity,
                                         bias=bo_sb[:, jt:jt + 1])
                else:
                    nc.vector.tensor_scalar(outT_sb[:, jt, :], ps[:, t, :],
                                            bo_sb[:, jt:jt + 1], None,
                                            ALU.add)
                nc.sync.dma_start(outT[jt * 128:(jt + 1) * 128, :],
                                  outT_sb[:, jt, :])

        def transposes_tail(b, eng):
            # tail transposes on the psO ring (slots free after normalize)
            tp = psO.tile([128, 4, 128], bf16, tag="psO", name=f"tpt{b}")
            for nc4 in range(4):
                nc.tensor.transpose(tp[:, nc4, :],
                                    OTn_sb[:, nc4, b, :, :], ident_sb[:])
            if eng is nc.scalar:
                eng.copy(OT_sb[:, 2 + b, :],
                         tp[:].rearrange("p a n -> p (a n)"))
            else:
                eng.tensor_copy(OT_sb[:, 2 + b, :],
                                tp[:].rearrange("p a n -> p (a n)"))

        pc0 = outproj_pair(0)
        pc1 = outproj_pair(1)
        normalize(1, o_ps1, (0,))
        normalize(1, o_ps1, (1,))
        transposes_tail(0, nc.vector)
        outproj_g(pc0, 0, 2, False)
        outproj_g(pc1, 1, 2, False)
        transposes_tail(1, nc.scalar)
        outproj_g(pc0, 0, 3, True)
        outproj_done(pc0, 0)
        outproj_g(pc1, 1, 3, True)
        outproj_done(pc1, 1)


# every TPB instruction encoding in this walrus build tolerates only a
# single semaphore wait -- split extras regardless of opcode
_NO_SPLIT_TYPES = {"InstEventSemaphore"}


def _split_matmul_waits(nc, mybir):
    """Several engine instruction encodings tolerate only one semaphore
    wait; move extra waits onto standalone single-wait EventSemaphore
    instructions inserted right before them on the same engine queue."""
    import bass_rust

    n = 0
    for bb in nc.m.functions[0].blocks:
        insts = list(bb.instructions)
        out = []
        changed = False
        for i in insts:
            si = i.sync_info
            if (type(i).__name__ not in _NO_SPLIT_TYPES and si is not None
                    and len(si.on_wait) > 1):
                w = list(si.on_wait)
                for wx in w[:-1]:
                    ev = mybir.InstEventSemaphore(name=f"mmw_{n}_{i.name}",
                                                  ins=[], outs=[])
                    ev.engine = i.engine
                    ev.sync_info = bass_rust.SyncInfo(on_wait=[wx],
                                                      on_update=[])
                    out.append(ev)
                    n += 1
                si.on_wait = [w[-1]]
                changed = True
            out.append(i)
        if changed:
            bb.instructions = out


def _host_prep(inputs):
    x_q = np.asarray(inputs["x_q"], np.float32)
    x_k = np.asarray(inputs["x_k"], np.float32)
    x_v = np.asarray(inputs["x_v"], np.float32)
    attn_mask = np.asarray(inputs["attn_mask"]).astype(bool)
    kpm = np.asarray(inputs["key_padding_mask"]).astype(bool)
    Wq = np.asarray(inputs["Wq"], np.float32)
    Wk = np.asarray(inputs["Wk"], np.float32)
    Wv = np.asarray(inputs["Wv"], np.float32)
    Wo = np.asarray(inputs["Wo"], np.float32)
    bq = np.asarray(inputs["bq"], np.float32)
    bv = np.asarray(inputs["bv"], np.float32)
    bo = np.asarray(inputs["bo"], np.float32)
    mw = np.asarray(inputs["mask_weight"], np.float64)

    # c[h,r] = softmax(mask_weight[h,:R]) * mask_weight[h,R]
    e = np.exp(mw[:, :R] - mw[:, :R].max(axis=1, keepdims=True))
    w = e / e.sum(axis=1, keepdims=True)
    c = (w * mw[:, R:R + 1]).astype(np.float32)          # [H, R]

    # active key tiles (at least one unpadded key) -- shared across batch
    tile_padded = kpm.reshape(B, MT, 128).all(axis=2)    # [B, MT]
    active = [mt for mt in range(MT) if not tile_padded[:, mt].all()]
    MTA = len(active)

    scale = np.float32(1.0 / np.sqrt(C))
    wq_s = (Wq * scale).astype(np.float32)
    bq_s = (bq * scale).astype(np.float32)
    bo_p = (bv @ Wo + bo).astype(np.float32)

    bq2 = np.ascontiguousarray(bq_s.reshape(4, 128).T)
    bo2 = np.ascontiguousarray(bo_p.reshape(4, 128).T)

    bf = ml_dtypes.bfloat16
    common = dict(wq=wq_s.astype(bf), wk=Wk.astype(bf), wv=Wv.astype(bf),
                  wo=Wo.astype(bf), bq2=bq2, bo2=bo2,
                  ident=np.eye(128, dtype=bf))

    emul = np.exp(-c)                                    # [H, R] in (0,1]
    in_maps = []
    for core in range(NCORES):
        b, ns = core // 4, core % 4
        n0 = ns * NS
        pad01 = (~kpm[b]).astype(np.float32)             # [N]
        pad2 = np.ascontiguousarray(
            pad01.reshape(MT, 128).T[:, active])         # [128, MTA]
        pad8 = np.ascontiguousarray(np.repeat(pad2[:, :, None], H, axis=2))
        inv = attn_mask[b, :, n0:n0 + NS, :]             # [R, NS, N]
        ep = np.empty((H // 2, 128, MTA, 2, NS), bf)
        for mi, mt in enumerate(active):
            invt = inv[:, :, mt * 128:(mt + 1) * 128]    # [R, NS, 128]
            bias = np.einsum('hr,rnm->hmn', c, invt.astype(np.float32))
            ep[:, :, mi] = np.exp(-bias).astype(bf).reshape(
                H // 2, 2, 128, NS).transpose(0, 2, 1, 3)
        ep = np.ascontiguousarray(ep)
        m = dict(common)
        m["xtq"] = np.ascontiguousarray(x_q[n0:n0 + NS, b, :].T).astype(bf)
        m["xtk"] = np.ascontiguousarray(x_k[:, b, :].T).astype(bf)
        m["xtv"] = np.ascontiguousarray(x_v[:, b, :].T).astype(bf)
        m["epl"] = ep
        m["pad"] = pad2
        m["pad8"] = pad8
        in_maps.append(m)
    return in_maps, active


def kernel(**inputs) -> np.ndarray:
    from concourse.bass_utils import run_bass_kernel_spmd

    in_maps, active = _host_prep(inputs)
    key = tuple(active)
    if key not in _cache:
        _cache[key] = _build_program(active)
        _cache["nc"] = _cache[key]
    nc = _cache[key]

    res = run_bass_kernel_spmd(nc, in_maps, list(range(NCORES)))

    out = np.empty((N, B, D), np.float32)
    for core in range(NCORES):
        b, ns = core // 4, core % 4
        n0 = ns * NS
        out[n0:n0 + NS, b, :] = res.results[core]["outT"].T.astype(np.float32)
    return out


# revision 7
# speedup vs baseline: 1.0297x; 1.0060x over previous
"""Bass/Trainium2 kernel for DynamicMultiheadAttention (sparse_attention).

v2: attention@V runs in TRANSPOSED orientation — oT[n, c] = p2.T @ v with
p2 (scores) as the PE stationary and v ([128, 65] incl. ones column) as the
moving tensor.  PE matmul cost is output-free-size cycles, so the o-path
drops from 120x[65,512] (213ns) to 480x[128,65] (27ns): ~-12.6us PE.
The softmax rowsum lands as PSUM column 64 per (n-partition, head), so
normalization is a per-partition tensor_scalar DIVIDE (no reciprocal
broadcasts, no ones2 matmuls, no ACT copies).  The normalized oT is
PE-transposed back ([n,dh] -> [dh,n], 16x 53ns) to feed the output
projection, which is unchanged.

Sharding: 8 cores = (batch b in {0,1}) x (query-slice of 512 rows).
Each core computes all 8 heads for its (b, n-slice); scores sT[m, n]
with keys m on partitions.

The relative-mask bias is applied multiplicatively after the exp:
  exp(s + rel) = exp(s) * E,  E[h,m,n] = exp(-sum_r c[h,r]*attn_mask)
E is precomputed on the host as bf16 planes; the all-16-bit
tensor_tensor multiply runs in the DVE 2x_1p fast mode (~593ns per
two-head tile).

Fully-padded key tiles are skipped at program-build time; partially
padded tiles are handled by zeroing the affected rows of V and of the
appended ones-column.  The k bias is softmax-invariant and dropped; the
v bias folds into the output bias: bo' = bv @ Wo + bo.

Every TPB instruction encoding in this walrus build tolerates only ONE
semaphore wait; a post-pass (_split_matmul_waits) moves extra waits onto
standalone single-wait EventSemaphore instructions.
"""

import numpy as np
import ml_dtypes
import os

def _B(name, default):
    return int(os.environ.get("KB_" + name, default))

N, B, D = 2048, 2, 512
H, R = 8, 3
C = D // H          # 64
NS = N // 4         # 512 query rows per core
NCORES = 8
MT = N // 128       # 16 key tiles

_cache = {}


def _build_program(active, reps=1):
    import concourse.bass as bass
    import concourse.mybir as mybir
    import concourse.tile as tile
    from contextlib import ExitStack

    f32 = mybir.dt.float32
    f32r = mybir.dt.float32r
    bf16 = mybir.dt.bfloat16
    u8 = mybir.dt.uint8
    AFT = mybir.ActivationFunctionType
    ALU = mybir.AluOpType

    MTA = len(active)

    nc = bass.Bass()

    xtq = nc.declare_dram_parameter("xtq", [D, NS], bf16, isOutput=False)
    xtk = nc.declare_dram_parameter("xtk", [D, N], bf16, isOutput=False)
    xtv = nc.declare_dram_parameter("xtv", [D, N], bf16, isOutput=False)
    # E planes, partition-major: [hp, p(=m%128), mi, j(head in pair), n]
    epl = nc.declare_dram_parameter("epl", [H // 2, 128, MTA, 2, NS], bf16,
                                    isOutput=False)
    wq = nc.declare_dram_parameter("wq", [D, D], bf16, isOutput=False)
    wk = nc.declare_dram_parameter("wk", [D, D], bf16, isOutput=False)
    wv = nc.declare_dram_parameter("wv", [D, D], bf16, isOutput=False)
    wo = nc.declare_dram_parameter("wo", [D, D], bf16, isOutput=False)
    bq2 = nc.declare_dram_parameter("bq2", [128, 4], f32, isOutput=False)
    bo2 = nc.declare_dram_parameter("bo2", [128, 4], f32, isOutput=False)
    # per-active-tile pad multiplier planes (zero padded key rows of V)
    pad = nc.declare_dram_parameter("pad", [128, MTA], f32, isOutput=False)
    pad8 = nc.declare_dram_parameter("pad8", [128, MTA, H], f32, isOutput=False)
    ident = nc.declare_dram_parameter("ident", [128, 128], bf16, isOutput=False)
    outT = nc.declare_dram_parameter("outT", [D, NS], bf16, isOutput=True)

    with tile.TileContext(nc) as tc, ExitStack() as ctx:
        mm = nc.tensor.matmul
        _run_once(nc, tc, ctx, mm, tile, mybir, f32, f32r, bf16, u8,
                  AFT, ALU, active, xtq, xtk, xtv, epl, wq, wk, wv, wo,
                  bq2, bo2, pad, pad8, ident, outT)

    _split_matmul_waits(nc, mybir)
    return nc


def _run_once(nc, tc, ctx, mm, tile, mybir, f32, f32r, bf16, u8, AFT, ALU,
              active, xtq, xtk, xtv, epl, wq, wk, wv, wo, bq2, bo2,
              pad, pad8, ident, outT):
    from contextlib import ExitStack
    MTA = len(active)
    with ExitStack() as ctx:
        const_pool = ctx.enter_context(tc.tile_pool(name="const", bufs=1))
        persist = ctx.enter_context(tc.tile_pool(name="persist", bufs=1))

        # constants ride the Pool queue: SP's 650ns-per-DMA dispatch rate is
        # the lead-in bottleneck, so it is reserved for the q/k-path inputs
        loads = []
        bq_sb = const_pool.tile([128, 4], f32)
        loads.append(nc.gpsimd.dma_start(bq_sb[:], bq2[:]))
        bo_sb = const_pool.tile([128, 4], f32)
        loads.append(nc.gpsimd.dma_start(bo_sb[:], bo2[:]))
        pad_sb = const_pool.tile([128, MTA], f32)
        loads.append(nc.gpsimd.dma_start(pad_sb[:], pad[:]))
        pad8_sb = const_pool.tile([128, MTA, H], f32)
        loads.append(nc.gpsimd.dma_start(pad8_sb[:], pad8[:]))
        ident_sb = const_pool.tile([128, 128], bf16)
        loads.append(nc.gpsimd.dma_start(ident_sb[:], ident[:]))
        wo_sb = persist.tile([128, 4, D], bf16)

        kT_sb = persist.tile([128, 4, N], bf16)
        qT_sb = persist.tile([128, 4, NS], bf16)
        v_sb = persist.tile([128, MTA, H, C + 1], bf16)
        OT_sb = persist.tile([128, 4, NS], bf16)
        # normalized oT per pass: [n, nc4, dh-block, head-in-block, c]
        OTn_sb = persist.tile([128, 4, 2, 2, C], bf16)
        outT_sb = persist.tile([128, 4, NS], bf16)

        # ---- Phase A (part 1): DMAs + projections needed by pass 0 ----
        xw_pool = ctx.enter_context(tc.tile_pool(name="xw", bufs=1))
        wq_sb = xw_pool.tile([128, 4, D], bf16, tag="w")
        wk_sb = xw_pool.tile([128, 4, D], bf16, tag="w2")
        wv_sb = xw_pool.tile([128, 4, D], bf16, tag="w3")
        xtq_sb = xw_pool.tile([128, 4, NS], bf16, tag="xq")
        xtk_sb = xw_pool.tile([128, 4, N], bf16, tag="xk")
        xtv_sb = xw_pool.tile([128, 4, N], bf16, tag="xv")
        # q/k path on SP, v path on the Pool queue: parallel dispatch halves
        # the dispatch-bound lead-in.
        nc.sync.dma_start(wq_sb[:, :, 0:256],
                          wq[:, 0:256].rearrange("(c p) d -> p c d", p=128))
        nc.sync.dma_start(xtq_sb[:, 0:2, :],
                          xtq[0:256].rearrange("(c p) n -> p c n", p=128))
        nc.sync.dma_start(xtq_sb[:, 2:4, :],
                          xtq[256:512].rearrange("(c p) n -> p c n", p=128))
        nc.sync.dma_start(wk_sb[:, :, 0:256],
                          wk[:, 0:256].rearrange("(c p) d -> p c d", p=128))
        nc.gpsimd.dma_start(wv_sb[:],
                            wv[:].rearrange("(c p) d -> p c d", p=128))
        for mb in range(4):
            sl = slice(mb * 512, (mb + 1) * 512)
            nc.sync.dma_start(
                xtk_sb[:, :, sl],
                xtk[:, sl].rearrange("(kc p) m -> p kc m", p=128))
            nc.gpsimd.dma_start(
                xtv_sb[:, :, sl],
                xtv[:, sl].rearrange("(kc p) m -> p kc m", p=128))

        nc.sync.dma_start(wq_sb[:, :, 256:512],
                          wq[:, 256:512].rearrange("(c p) d -> p c d", p=128))
        nc.sync.dma_start(wk_sb[:, :, 256:512],
                          wk[:, 256:512].rearrange("(c p) d -> p c d", p=128))

        # E-plane tiles: quarter planes [128, 4, 2, NS] bf16 per fetch
        ep_pool = ctx.enter_context(tc.tile_pool(name="ep", bufs=_B("EP", 6)))
        pT_pool = ctx.enter_context(tc.tile_pool(name="pT", bufs=_B("PT", 5)))
        p2_pool = ctx.enter_context(tc.tile_pool(name="p2", bufs=_B("P2", 5)))
        EH = 4
        ep_tiles = {}

        def fetch_e(hp, q, eng=None):
            h0 = q * EH
            hn = min(EH, MTA - h0)
            t = ep_pool.tile([128, EH, 2, NS], bf16, tag="ep")
            (eng or nc.gpsimd).dma_start(t[:, 0:hn, :, :],
                                         epl[hp, :, h0:h0 + hn, :, :])
            ep_tiles[(hp, q)] = t

        fetch_e(0, 0)
        fetch_e(1, 0)
        fetch_e(0, 1)
        fetch_e(1, 1)
        fetch_e(0, 2)
        fetch_e(1, 2)

        vones = [nc.vector.tensor_copy(
            v_sb[:, :, :, C : C + 1],
            pad8_sb[:, :, :].rearrange("p m (h o) -> p m h o", o=1))]

        with tc.tile_pool(name="psA", bufs=_B("PSA", 8), space="PSUM") as psA:
            # qT[dh, n] = (Wq/8).T @ xT_q  (+ bq/8 per-partition), heads 0-3
            for j in range(2):
                ps = psA.tile([128, NS], f32, tag="psA")
                for kc in range(4):
                    mm(ps[:], wq_sb[:, kc, j * 128:(j + 1) * 128],
                       xtq_sb[:, kc, :], start=(kc == 0), stop=(kc == 3))
                nc.scalar.activation(qT_sb[:, j, :], ps[:], AFT.Identity,
                                     bias=bq_sb[:, j:j + 1])

            # kT[dh, m] = Wk.T @ xT_k, heads 0-3 (k bias drops in softmax)
            for mb in range(4):
                for j in range(2):
                    ps = psA.tile([128, NS], f32, tag="psA")
                    for kc in range(4):
                        mm(ps[:], wk_sb[:, kc, j * 128:(j + 1) * 128],
                           xtk_sb[:, kc, mb * 512:(mb + 1) * 512],
                           start=(kc == 0), stop=(kc == 3))
                    if (mb + j) % 2 == 0:
                        nc.scalar.copy(kT_sb[:, j, mb * 512:(mb + 1) * 512],
                                       ps[:])
                    else:
                        nc.vector.tensor_copy(
                            kT_sb[:, j, mb * 512:(mb + 1) * 512], ps[:])

            # v[m, c] = xT_v.T @ Wv, padded key rows zeroed; evacuations
            # split DVE / ACT(scale) so they don't serialize the lead-in
            for mi, mt in enumerate(active):
                ps = psA.tile([128, D], f32, tag="psA")
                for kc in range(4):
                    mm(ps[:], xtv_sb[:, kc, mt * 128:(mt + 1) * 128],
                       wv_sb[:, kc, :], start=(kc == 0), stop=(kc == 3))
                if mi % 2 == 0:
                    nc.vector.tensor_scalar(
                        v_sb[:, mi, :, 0:C],
                        ps[:].rearrange("p (h c) -> p h c", h=H),
                        pad_sb[:, mi:mi + 1], None, ALU.mult)
                else:
                    nc.scalar.activation(
                        v_sb[:, mi, :, 0:C],
                        ps[:].rearrange("p (h c) -> p h c", h=H),
                        AFT.Identity, scale=pad_sb[:, mi:mi + 1])

        # PSUM pools for phase B (psA released its banks above)
        # psO-tag ring: 4 slots of 1 bank each.  Holds in turn: the o
        # accumulators ([128, 4, 65] f32: 4 heads + rowsum col per n-chunk),
        # the deferred j23 k projections ([128, 512] f32), and the output
        # projection tiles.  psS tiles are [128, 2, NS] (2 banks).
        small_pool = ctx.enter_context(tc.tile_pool(name="small", bufs=4))
        psO = ctx.enter_context(tc.tile_pool(name="psO", bufs=4, space="PSUM"))
        psS = ctx.enter_context(tc.tile_pool(name="psS", bufs=_B("PSS", 2), space="PSUM"))

        # ---- Phase B: attention, two passes of 4 heads (2 head pairs) ----
        def attn_pass(p, hooks={}):
            o_ps = [psO.tile([128, 4, C + 1], f32, tag="psO",
                             name=f"o_ps{p}_{i}") for i in range(4)]

            def emit_o(p2, mi, hpl):
                for j in range(2):
                    h = 4 * p + 2 * hpl + j
                    hl = 2 * hpl + j
                    for nc4 in range(4):
                        mm(o_ps[nc4][:, hl, :],
                           p2[:, j, nc4 * 128:(nc4 + 1) * 128],
                           v_sb[:, mi, h, :],
                           start=(mi == 0 and hl == 0),
                           stop=(mi == MTA - 1 and hl == 3),
                           skip_group_check=True)

            pending = []
            for mi in range(MTA):
                for fn in hooks.get(mi, ()):
                    fn()
                for hpl in range(2):
                    hp = 2 * p + hpl
                    if p == 0 and hpl == 0 and mi in (6, 8, 10, 12):
                        nq = {6: [(0, 3), (1, 3)], 8: [(2, 0), (3, 0)],
                              10: [(2, 1), (3, 1)], 12: [(2, 2), (3, 2)]}[mi]
                        for a, b in nq:
                            fetch_e(a, b)
                    if p == 1 and hpl == 0 and mi == 4:
                        fetch_e(2, 3)
                        fetch_e(3, 3)
                    s_ps = psS.tile([128, 2, NS], f32, tag="psS")
                    for j in range(2):
                        h = 4 * p + 2 * hpl + j
                        hj, ho = h // 2, (h % 2) * 64
                        mm(s_ps[:, j, :],
                           kT_sb[ho:ho + 64, hj, active[mi] * 128:active[mi] * 128 + 128],
                           qT_sb[ho:ho + 64, hj, :], start=True, stop=True)
                    pT = pT_pool.tile([128, 2, NS], bf16, tag="pT")
                    nc.scalar.activation(pT[:], s_ps[:], AFT.Exp)
                    p2 = p2_pool.tile([128, 2, NS], bf16, tag="p2")
                    # all-bf16 tensor_tensor: DVE 2x_1p fast mode (~593ns)
                    nc.vector.tensor_tensor(
                        p2[:], pT[:],
                        ep_tiles[(hp, mi // EH)][:, mi % EH, :, :], ALU.mult)
                    # transposed o accumulation (emit_o), two groups late
                    # so the in-order PE queue never blocks on the live
                    # exp+mult chain or the boundary normalize WAR.
                    pending.append((p2, mi, hpl))
                    if len(pending) > 2:
                        emit_o(*pending.pop(0))
            for args in pending:
                emit_o(*args)
            return o_ps

        def normalize(p, o_ps, halves=(0, 1), nc4s=(0, 1, 2, 3)):
            # oT[n, c] * (1/rowsum[n]): per-partition reciprocal (DVE) +
            # scalar multiply, split DVE/ACT (ACT is idle at pass ends;
            # tensor_scalar divide fails the walrus ISA check).
            for half in halves:
                for k, hl in enumerate((2 * half, 2 * half + 1)):
                    for nc4 in nc4s:
                        rec = small_pool.tile([128, 1], f32, tag="rec",
                                              name=f"rc{p}_{hl}_{nc4}")
                        nc.vector.reciprocal(rec[:], o_ps[nc4][:, hl, C:C + 1])
                        if (nc4 + k) % 2 == 0:
                            nc.vector.tensor_scalar(
                                OTn_sb[:, nc4, half, hl % 2, :],
                                o_ps[nc4][:, hl, 0:C],
                                rec[:], None, ALU.mult)
                        else:
                            nc.scalar.activation(
                                OTn_sb[:, nc4, half, hl % 2, :],
                                o_ps[nc4][:, hl, 0:C],
                                AFT.Identity, scale=rec[:])

        def transposes(p, b, eng):
            # [n, dh-block] -> [dh-block, n] via PE transpose, then one
            # 512-wide evacuation copy into OT_sb[:, 2p+b, :].
            tp = psS.tile([128, 4, 128], bf16, tag="psS", name=f"tp{p}_{b}")
            for nc4 in range(4):
                nc.tensor.transpose(tp[:, nc4, :], OTn_sb[:, nc4, b, :, :],
                                    ident_sb[:])
            if eng is nc.scalar:
                eng.copy(OT_sb[:, 2 * p + b, :],
                         tp[:].rearrange("p a n -> p (a n)"))
            else:
                eng.tensor_copy(OT_sb[:, 2 * p + b, :],
                                tp[:].rearrange("p a n -> p (a n)"))

        # deferred projections for heads 4-7 (j-blocks 2,3)
        def proj_j23_k_hook(mb):
            ps = psS.tile([128, 2, NS], f32, tag="psS", name=f"kh{mb}")
            for j in (2, 3):
                for kc in range(4):
                    mm(ps[:, j - 2, :], wk_sb[:, kc, j * 128:(j + 1) * 128],
                       xtk_sb[:, kc, mb * 512:(mb + 1) * 512],
                       start=(kc == 0), stop=(kc == 3))
            nc.vector.tensor_copy(
                kT_sb[:, 2, mb * 512:(mb + 1) * 512], ps[:, 0, :])
            nc.vector.tensor_copy(
                kT_sb[:, 3, mb * 512:(mb + 1) * 512], ps[:, 1, :])

        def proj_j23_k_psS(mb):
            ps = psS.tile([128, 2, NS], f32, tag="psS", name=f"kp{mb}")
            for j in (2, 3):
                for kc in range(4):
                    mm(ps[:, j - 2, :], wk_sb[:, kc, j * 128:(j + 1) * 128],
                       xtk_sb[:, kc, mb * 512:(mb + 1) * 512],
                       start=(kc == 0), stop=(kc == 3))
            nc.scalar.copy(kT_sb[:, 2, mb * 512:(mb + 1) * 512], ps[:, 0, :])
            nc.vector.tensor_copy(
                kT_sb[:, 3, mb * 512:(mb + 1) * 512], ps[:, 1, :])

        def proj_j23_q_psS():
            ps = psS.tile([128, 2, NS], f32, tag="psS", name="qp23")
            for j in (2, 3):
                for kc in range(4):
                    mm(ps[:, j - 2, :], wq_sb[:, kc, j * 128:(j + 1) * 128],
                       xtq_sb[:, kc, :], start=(kc == 0), stop=(kc == 3))
            for j in (2, 3):
                nc.scalar.activation(qT_sb[:, j, :], ps[:, j - 2, :],
                                     AFT.Identity, bias=bq_sb[:, j:j + 1])

        def proj_j23_k_psO(mb):
            for j in (2, 3):
                ps = psO.tile([128, NS], f32, tag="psO", name=f"kp{mb}_{j}")
                for kc in range(4):
                    mm(ps[:], wk_sb[:, kc, j * 128:(j + 1) * 128],
                       xtk_sb[:, kc, mb * 512:(mb + 1) * 512],
                       start=(kc == 0), stop=(kc == 3))
                if j == 2:
                    nc.scalar.copy(
                        kT_sb[:, j, mb * 512:(mb + 1) * 512], ps[:])
                else:
                    nc.vector.tensor_copy(
                        kT_sb[:, j, mb * 512:(mb + 1) * 512], ps[:])

        o_ps0 = attn_pass(0)
        proj_j23_k_psS(0)
        proj_j23_q_psS()
        proj_j23_k_psS(1)
        # both normalize halves must finish reading the o banks before any
        # psO-ring reuse: each bank holds all four heads of the pass.
        normalize(0, o_ps0, (0,))
        normalize(0, o_ps0, (1,))
        transposes(0, 0, nc.vector)
        transposes(0, 1, nc.scalar)
        proj_j23_k_psO(2)
        proj_j23_k_psO(3)
        for c in range(4):
            nc.sync.dma_start(wo_sb[:, c, :], wo[c * 128:(c + 1) * 128, :])

        o_ps1 = attn_pass(1)

        # ---- Phase C: output projection, pipelined with pass-1 tail ----
        def outproj_partial(jt):
            ps = psO.tile([128, NS], f32, tag="psO", name=f"oc{jt}")
            for g in (0, 1, 2):
                mm(ps[:], wo_sb[:, g, jt * 128:(jt + 1) * 128],
                   OT_sb[:, g, :], start=(g == 0), stop=False)
            return ps

        def outproj_finish(jt, ps=None):
            if ps is None:
                ps = psO.tile([128, NS], f32, tag="psO", name=f"oc{jt}")
                for g in (0, 1, 2):
                    mm(ps[:], wo_sb[:, g, jt * 128:(jt + 1) * 128],
                       OT_sb[:, g, :], start=(g == 0), stop=False)
            mm(ps[:], wo_sb[:, 3, jt * 128:(jt + 1) * 128],
               OT_sb[:, 3, :], start=False, stop=True)
            if jt % 2 == 0:
                nc.scalar.activation(outT_sb[:, jt, :], ps[:], AFT.Identity,
                                     bias=bo_sb[:, jt:jt + 1])
            else:
                nc.vector.tensor_scalar(outT_sb[:, jt, :], ps[:],
                                        bo_sb[:, jt:jt + 1], None, ALU.add)
            nc.sync.dma_start(outT[jt * 128:(jt + 1) * 128, :],
                              outT_sb[:, jt, :])

        # tail: g0/g1 out-projection partials run on psS tiles BEFORE
        # normalize(1) (they only need OT hj0/hj1, ready since pass 0);
        # the tail transposes move to the psO ring (post-normalize slots).
        def outproj_pair(pair):
            ps = psS.tile([128, 2, NS], f32, tag="psS", name=f"ocp{pair}")
            for t in range(2):
                jt = 2 * pair + t
                for g in (0, 1):
                    mm(ps[:, t, :], wo_sb[:, g, jt * 128:(jt + 1) * 128],
                       OT_sb[:, g, :], start=(g == 0), stop=False)
            return ps

        def outproj_g(ps, pair, g, stop):
            for t in range(2):
                jt = 2 * pair + t
                mm(ps[:, t, :], wo_sb[:, g, jt * 128:(jt + 1) * 128],
                   OT_sb[:, g, :], start=False, stop=stop)

        def outproj_done(ps, pair):
            for t in range(2):
                jt = 2 * pair + t
                if jt % 2 == 0:
                    nc.scalar.activation(outT_sb[:, jt, :], ps[:, t, :],
                                         AFT.Identity,
                                         bias=bo_sb[:, jt:jt + 1])
                else:
                    nc.vector.tensor_scalar(outT_sb[:, jt, :], ps[:, t, :],
                                            bo_sb[:, jt:jt + 1], None,
                                            ALU.add)
            # one consolidated output DMA per pair: halves the SP dispatch
            # and DMA-semaphore serialization at the very end of the tail
            nc.sync.dma_start(
                outT[pair * 256:(pair + 1) * 256, :].rearrange(
                    "(c p) n -> p c n", p=128),
                outT_sb[:, 2 * pair:2 * pair + 2, :])

        def transposes_tail(b, eng):
            # tail transposes on the psO ring (slots free after normalize)
            tp = psO.tile([128, 4, 128], bf16, tag="psO", name=f"tpt{b}")
            for nc4 in range(4):
                nc.tensor.transpose(tp[:, nc4, :],
                                    OTn_sb[:, nc4, b, :, :], ident_sb[:])
            if eng is nc.scalar:
                eng.copy(OT_sb[:, 2 + b, :],
                         tp[:].rearrange("p a n -> p (a n)"))
            else:
                eng.tensor_copy(OT_sb[:, 2 + b, :],
                                tp[:].rearrange("p a n -> p (a n)"))

        pc0 = outproj_pair(0)
        pc1 = outproj_pair(1)
        normalize(1, o_ps1, (0,))
        normalize(1, o_ps1, (1,))
        transposes_tail(0, nc.vector)
        outproj_g(pc0, 0, 2, False)
        outproj_g(pc1, 1, 2, False)
        transposes_tail(1, nc.vector)
        outproj_g(pc0, 0, 3, True)
        outproj_done(pc0, 0)
        outproj_g(pc1, 1, 3, True)
        outproj_done(pc1, 1)


# every TPB instruction encoding in this walrus build tolerates only a
# single semaphore wait -- split extras regardless of opcode
_NO_SPLIT_TYPES = {"InstEventSemaphore"}


def _split_matmul_waits(nc, mybir):
    """Several engine instruction encodings tolerate only one semaphore
    wait; move extra waits onto standalone single-wait EventSemaphore
    instructions inserted right before them on the same engine queue."""
    import bass_rust

    n = 0
    for bb in nc.m.functions[0].blocks:
        insts = list(bb.instructions)
        out = []
        changed = False
        for i in insts:
            si = i.sync_info
            if (type(i).__name__ not in _NO_SPLIT_TYPES and si is not None
                    and len(si.on_wait) > 1):
                w = list(si.on_wait)
                for wx in w[:-1]:
                    ev = mybir.InstEventSemaphore(name=f"mmw_{n}_{i.name}",
                                                  ins=[], outs=[])
                    ev.engine = i.engine
                    ev.sync_info = bass_rust.SyncInfo(on_wait=[wx],
                                                      on_update=[])
                    out.append(ev)
                    n += 1
                si.on_wait = [w[-1]]
                changed = True
            out.append(i)
        if changed:
            bb.instructions = out


def _host_prep(inputs):
    x_q = np.asarray(inputs["x_q"], np.float32)
    x_k = np.asarray(inputs["x_k"], np.float32)
    x_v = np.asarray(inputs["x_v"], np.float32)
    attn_mask = np.asarray(inputs["attn_mask"]).astype(bool)
    kpm = np.asarray(inputs["key_padding_mask"]).astype(bool)
    Wq = np.asarray(inputs["Wq"], np.float32)
    Wk = np.asarray(inputs["Wk"], np.float32)
    Wv = np.asarray(inputs["Wv"], np.float32)
    Wo = np.asarray(inputs["Wo"], np.float32)
    bq = np.asarray(inputs["bq"], np.float32)
    bv = np.asarray(inputs["bv"], np.float32)
    bo = np.asarray(inputs["bo"], np.float32)
    mw = np.asarray(inputs["mask_weight"], np.float64)

    # c[h,r] = softmax(mask_weight[h,:R]) * mask_weight[h,R]
    e = np.exp(mw[:, :R] - mw[:, :R].max(axis=1, keepdims=True))
    w = e / e.sum(axis=1, keepdims=True)
    c = (w * mw[:, R:R + 1]).astype(np.float32)          # [H, R]

    # active key tiles (at least one unpadded key) -- shared across batch
    tile_padded = kpm.reshape(B, MT, 128).all(axis=2)    # [B, MT]
    active = [mt for mt in range(MT) if not tile_padded[:, mt].all()]
    MTA = len(active)

    scale = np.float32(1.0 / np.sqrt(C))
    wq_s = (Wq * scale).astype(np.float32)
    bq_s = (bq * scale).astype(np.float32)
    bo_p = (bv @ Wo + bo).astype(np.float32)

    bq2 = np.ascontiguousarray(bq_s.reshape(4, 128).T)
    bo2 = np.ascontiguousarray(bo_p.reshape(4, 128).T)

    bf = ml_dtypes.bfloat16
    common = dict(wq=wq_s.astype(bf), wk=Wk.astype(bf), wv=Wv.astype(bf),
                  wo=Wo.astype(bf), bq2=bq2, bo2=bo2,
                  ident=np.eye(128, dtype=bf))

    emul = np.exp(-c)                                    # [H, R] in (0,1]
    in_maps = []
    for core in range(NCORES):
        b, ns = core // 4, core % 4
        n0 = ns * NS
        pad01 = (~kpm[b]).astype(np.float32)             # [N]
        pad2 = np.ascontiguousarray(
            pad01.reshape(MT, 128).T[:, active])         # [128, MTA]
        pad8 = np.ascontiguousarray(np.repeat(pad2[:, :, None], H, axis=2))
        inv = attn_mask[b, :, n0:n0 + NS, :]             # [R, NS, N]
        ep = np.empty((H // 2, 128, MTA, 2, NS), bf)
        for mi, mt in enumerate(active):
            invt = inv[:, :, mt * 128:(mt + 1) * 128]    # [R, NS, 128]
            bias = np.einsum('hr,rnm->hmn', c, invt.astype(np.float32))
            ep[:, :, mi] = np.exp(-bias).astype(bf).reshape(
                H // 2, 2, 128, NS).transpose(0, 2, 1, 3)
        ep = np.ascontiguousarray(ep)
        m = dict(common)
        m["xtq"] = np.ascontiguousarray(x_q[n0:n0 + NS, b, :].T).astype(bf)
        m["xtk"] = np.ascontiguousarray(x_k[:, b, :].T).astype(bf)
        m["xtv"] = np.ascontiguousarray(x_v[:, b, :].T).astype(bf)
        m["epl"] = ep
        m["pad"] = pad2
        m["pad8"] = pad8
        in_maps.append(m)
    return in_maps, active


def kernel(**inputs) -> np.ndarray:
    from concourse.bass_utils import run_bass_kernel_spmd

    in_maps, active = _host_prep(inputs)
    key = tuple(active)
    if key not in _cache:
        _cache[key] = _build_program(active)
        _cache["nc"] = _cache[key]
    nc = _cache[key]

    res = run_bass_kernel_spmd(nc, in_maps, list(range(NCORES)))

    out = np.empty((N, B, D), np.float32)
    for core in range(NCORES):
        b, ns = core // 4, core % 4
        n0 = ns * NS
        out[n0:n0 + NS, b, :] = res.results[core]["outT"].T.astype(np.float32)
    return out
